# revision 9
# baseline (speedup 1.0000x reference)
"""Trainium2 Bass kernel for nn_Chambers: 6 per-chamber MLPs over a shared
reservoir input, followed by 5 coupled-chamber fixed-point iterations.

Data-parallel over 8 NeuronCores: each core processes B/8 = 32768 rows.

Per-core pipeline (feature-major MLP, batch-major coupling):
  - res [R,100] loaded in [128,100] row tiles, transposed on the PE
    (is_transpose matmul vs identity) into resT [100, N] in PSUM, copied
    to SBUF by the DVE.
  - L1 (K=100,M=128) per chamber; silu+bias fused into one ACT op
    reading PSUM (bias is a per-partition AP).
  - L2 (K=128,M=64): two chambers run concurrently via column tiling
    (tile_position (0,0)/(0,64)), one [128,N] PSUM tile -> one silu op.
  - L3 (K=64,M=32): six chambers concurrent via row+column tiling.
  - L4: per-chamber dot products are K-stacked into two accumulating
    matmuls (block-column lhsT) producing raw = z4 as a [6,N] PSUM tile.
  - raw is DMA-scattered into batch-major [128, 6*F] tiles; sigmoid is
    computed as 0.5+0.5*tanh(0.5*x+0.5*b4) so every ACT function used
    (Silu/Tanh/Sin) lives in the single `silu_and_others` table set.
  - 5 coupling iterations run on the DVE using sin/cos expansion:
      delta_i = K*( cos a_i * (C sin a)_i - sin a_i * (C cos a)_i )
    with the 6x6 symmetric matvec done as 15 paired
    scalar_tensor_tensor AXPYs ([2,F] strided block-pair APs).
  - Coupling runs per batch slice so it overlaps the next slice's MLP.

Outputs: act (post-coupling) and raw (z4; b4 added on host).
"""

import numpy as np

# ---- problem constants (fixed by the task; kernel.py must be self-contained)
B = 262144
RES_DIM = 100
NCH = 6
CF_ITERS = 5
CF_K = 0.02
DECAY = np.array([0.9, 0.93, 0.85, 0.97, 0.88, 0.94], dtype=np.float32)
COUPLING = np.array([
    [0.0, -0.3, 0.6, 0.4, -0.2, 0.3],
    [-0.3, 0.0, -0.5, -0.7, 0.6, 0.4],
    [0.6, -0.5, 0.0, 0.3, -0.3, 0.2],
    [0.4, -0.7, 0.3, 0.0, -0.4, 0.5],
    [-0.2, 0.6, -0.3, -0.4, 0.0, 0.3],
    [0.3, 0.4, 0.2, 0.5, 0.3, 0.0]], dtype=np.float32)
N_CORES = 8
R_CORE = B // N_CORES          # 32768 rows per core
CHUNK = 1024                   # rows per MLP chunk
HALF_PI = float(np.pi / 2.0)

_BUILD_CACHE = {}


def _build(R, slice_rows):
    """Emit + compile the per-core SPMD program for R rows, coupling in
    slices of slice_rows. Returns the compiled Bacc object."""
    from contextlib import ExitStack
    import concourse.bass as bass
    import concourse.mybir as mybir
    from concourse import bacc, tile, masks

    f32 = mybir.dt.float32
    AF = mybir.ActivationFunctionType
    OP = mybir.AluOpType

    assert R % slice_rows == 0 and slice_rows % CHUNK == 0
    n_slices = R // slice_rows
    chunks_per_slice = slice_rows // CHUNK
    F = slice_rows // 128          # free cols per chamber in coupling tiles
    PPC = CHUNK // F               # partitions covered by one chunk's scatter
    KC = (CF_K * COUPLING).astype(np.float64)
    # (i,j) pairs: 3 "init" pairs covering each block once, 12 accumulating
    init_pairs = [(0, 1), (2, 3), (4, 5)]
    rest_pairs = [(i, j) for i in range(6) for j in range(i + 1, 6)
                  if (i, j) not in init_pairs]

    nc = bacc.Bacc("TRN2", target_bir_lowering=False, debug=False,
                   num_devices=N_CORES)
    res = nc.dram_tensor("res", [R, RES_DIM], f32, kind="ExternalInput").ap()
    w1t = nc.dram_tensor("w1t", [RES_DIM, 6 * 128], f32, kind="ExternalInput").ap()
    b1t = nc.dram_tensor("b1t", [128, 6], f32, kind="ExternalInput").ap()
    w2t = nc.dram_tensor("w2t", [128, 6 * 64], f32, kind="ExternalInput").ap()
    b2p = nc.dram_tensor("b2p", [128, 3], f32, kind="ExternalInput").ap()
    # w3t holds W3^T twice (rows 0-63 and 64-127): row-tiled matmuls need
    # the stationary operand at the same base partition as the moving one
    w3t = nc.dram_tensor("w3t", [128, 6 * 32], f32, kind="ExternalInput").ap()
    b3a = nc.dram_tensor("b3a", [128, 1], f32, kind="ExternalInput").ap()
    b3b = nc.dram_tensor("b3b", [128, 1], f32, kind="ExternalInput").ap()
    w4f = nc.dram_tensor("w4f", [128, 12], f32, kind="ExternalInput").ap()
    # cst col 0 = pi/2 (cos bias); cols 1..6 = 0.5*b4[c] (tanh biases)
    cst = nc.dram_tensor("cst", [128, 8], f32, kind="ExternalInput").ap()
    act_o = nc.dram_tensor("act_o", [R, 6], f32, kind="ExternalOutput").ap()
    raw_o = nc.dram_tensor("raw_o", [R, 6], f32, kind="ExternalOutput").ap()

    def emit():
        with tile.TileContext(nc) as tc, ExitStack() as ctx:
            wp = ctx.enter_context(tc.tile_pool(name="w", bufs=1))
            t_w1t = wp.tile([RES_DIM, 6 * 128], f32, tag="w1t")
            nc.gpsimd.dma_start(t_w1t[:], w1t)
            t_b1t = wp.tile([128, 6], f32, tag="b1t")
            nc.gpsimd.dma_start(t_b1t[:], b1t)
            t_w2t = wp.tile([128, 6 * 64], f32, tag="w2t")
            nc.gpsimd.dma_start(t_w2t[:], w2t)
            t_b2p = wp.tile([128, 3], f32, tag="b2p")
            nc.gpsimd.dma_start(t_b2p[:], b2p)
            t_w3t = wp.tile([128, 6 * 32], f32, tag="w3t")
            nc.gpsimd.dma_start(t_w3t[:], w3t)
            t_b3a = wp.tile([128, 1], f32, tag="b3a")
            nc.gpsimd.dma_start(t_b3a[:], b3a)
            t_b3b = wp.tile([128, 1], f32, tag="b3b")
            nc.gpsimd.dma_start(t_b3b[:], b3b)
            t_w4f = wp.tile([128, 12], f32, tag="w4f")
            nc.gpsimd.dma_start(t_w4f[:], w4f)
            t_cst = wp.tile([128, 8], f32, tag="cst")
            nc.gpsimd.dma_start(t_cst[:], cst)
            t_id = wp.tile([128, 128], f32, tag="ident")
            masks.make_identity(nc, t_id[:])

            p_res = ctx.enter_context(tc.tile_pool(name="resin", bufs=3))
            p_ptr = ctx.enter_context(tc.tile_pool(name="ptr", bufs=2, space="PSUM"))
            p_rT = ctx.enter_context(tc.tile_pool(name="rT", bufs=2))
            p_mm = ctx.enter_context(tc.tile_pool(name="pmm", bufs=2, space="PSUM"))
            p_p4 = ctx.enter_context(tc.tile_pool(name="p4", bufs=1, space="PSUM"))
            p_h1 = ctx.enter_context(tc.tile_pool(name="h1", bufs=3))
            p_h2 = ctx.enter_context(tc.tile_pool(name="h2", bufs=4))
            p_h3 = ctx.enter_context(tc.tile_pool(name="h3", bufs=2))
            p_rsb = ctx.enter_context(tc.tile_pool(name="rsb", bufs=2))
            p_bm = ctx.enter_context(tc.tile_pool(name="bm", bufs=2))
            p_cpl = ctx.enter_context(tc.tile_pool(name="cpl", bufs=2))
            p_out = ctx.enter_context(tc.tile_pool(name="out", bufs=2))

            def blk(ap_t, c):
                return ap_t[:, c * F:(c + 1) * F]

            def pair_out(ap_t, i, j):
                x3 = ap_t[:].rearrange("p (c f) -> p c f", f=F)
                return x3[:, i:j + 1:(j - i), :]

            def pair_src(ap_t, i, j):
                # blocks [j, i] (swapped) via negative step
                x3 = ap_t[:].rearrange("p (c f) -> p c f", f=F)
                if i == 0:
                    return x3[:, j::-(j - i), :][:, 0:2, :]
                return x3[:, j:i - 1:-(j - i), :]

            for s in range(n_slices):
                rawbm = p_bm.tile([128, 6 * F], f32, tag="rawbm")
                for k in range(chunks_per_slice):
                    g = s * chunks_per_slice + k       # global chunk id
                    # -- load 1024 rows of res as 8 x [128,100] subtiles
                    rt = p_res.tile([128, 8 * RES_DIM], f32, tag="resin")
                    nc.gpsimd.dma_start(
                        rt[:].rearrange("p (j d) -> p j d", d=RES_DIM),
                        res[g * CHUNK:(g + 1) * CHUNK, :]
                        .rearrange("(j p) d -> p j d", p=128))
                    # -- transpose to resT [100, 1024] via PE
                    rT = p_rT.tile([RES_DIM, CHUNK], f32, tag="rT")
                    for h in range(2):
                        ptr = p_ptr.tile([RES_DIM, 512], f32, tag="ptr")
                        for j in range(4):
                            nc.tensor.transpose(
                                ptr[:, j * 128:(j + 1) * 128],
                                rt[:, (4 * h + j) * RES_DIM:(4 * h + j + 1) * RES_DIM],
                                t_id[:])
                        nc.vector.tensor_copy(rT[:, h * 512:(h + 1) * 512], ptr[:])

                    # -- L1 + L2 (paired)
                    h1s = {}
                    h2s = {}
                    for c in range(6):
                        ps1 = p_mm.tile([128, CHUNK], f32, tag="mm")
                        for h in range(2):
                            nc.tensor.matmul(
                                ps1[:, h * 512:(h + 1) * 512],
                                t_w1t[:, c * 128:(c + 1) * 128],
                                rT[:, h * 512:(h + 1) * 512])
                        h1 = p_h1.tile([128, CHUNK], f32, tag="h1")
                        nc.scalar.activation(h1[:], ps1[:], AF.Silu,
                                             bias=t_b1t[:, c:c + 1])
                        h1s[c] = h1
                        if c % 2 == 1:
                            p = c // 2
                            ps2 = p_mm.tile([128, CHUNK], f32, tag="mm")
                            for h in range(2):
                                nc.tensor.matmul(
                                    ps2[0:64, h * 512:(h + 1) * 512],
                                    t_w2t[:, (2 * p) * 64:(2 * p + 1) * 64],
                                    h1s[2 * p][:, h * 512:(h + 1) * 512],
                                    tile_position=(0, 0))
                                nc.tensor.matmul(
                                    ps2[64:128, h * 512:(h + 1) * 512],
                                    t_w2t[:, (2 * p + 1) * 64:(2 * p + 2) * 64],
                                    h1s[2 * p + 1][:, h * 512:(h + 1) * 512],
                                    tile_position=(0, 64))
                            h2 = p_h2.tile([128, CHUNK], f32, tag="h2")
                            nc.scalar.activation(h2[:], ps2[:], AF.Silu,
                                                 bias=t_b2p[:, p:p + 1])
                            h2s[p] = h2

                    # -- L3: chambers 0-3 -> ps3a rows 0..127; c5 -> ps3b rows
                    # 64..95, c4 -> ps3b rows 96..127 (row base for K=64 must
                    # be 0/64; all six (row,col) subarray sets stay distinct)
                    ps3a = p_mm.tile([128, CHUNK], f32, tag="mm")
                    ps3b = p_p4.tile([128, CHUNK], f32, tag="p4")
                    # (chamber, h2 pair, rows(0=upper 64,1=lower), out tile, out row base)
                    l3 = [(0, 0, 0, ps3a, 0), (1, 0, 1, ps3a, 32),
                          (2, 1, 0, ps3a, 64), (3, 1, 1, ps3a, 96),
                          (4, 2, 0, ps3b, 96), (5, 2, 1, ps3b, 64)]
                    for (c, p, half, pst, rb) in l3:
                        for h in range(2):
                            nc.tensor.matmul(
                                pst[rb:rb + 32, h * 512:(h + 1) * 512],
                                t_w3t[64 * half:64 * half + 64, c * 32:(c + 1) * 32],
                                h2s[p][64 * half:64 * half + 64,
                                       h * 512:(h + 1) * 512],
                                tile_position=(64 * half, rb))
                    h3a = p_h3.tile([128, CHUNK], f32, tag="h3a")
                    nc.scalar.activation(h3a[:], ps3a[:], AF.Silu, bias=t_b3a[:])
                    h3b = p_h3.tile([128, CHUNK], f32, tag="h3b")
                    nc.scalar.activation(h3b[64:128, :], ps3b[64:128, :], AF.Silu,
                                         bias=t_b3b[64:128, :])

                    # -- L4: raw[0:6] = w4f.T @ h3 (K-stacked accumulation)
                    for h in range(2):
                        nc.tensor.matmul(
                            ps3b[0:6, h * 512:(h + 1) * 512],
                            t_w4f[:, 0:6],
                            h3a[:, h * 512:(h + 1) * 512],
                            start=True, stop=False, tile_position=(0, 0))
                        nc.tensor.matmul(
                            ps3b[0:6, h * 512:(h + 1) * 512],
                            t_w4f[64:128, 6:12],
                            h3b[64:128, h * 512:(h + 1) * 512],
                            start=False, stop=True, tile_position=(64, 0))
                    rsb = p_rsb.tile([6, CHUNK], f32, tag="rsb")
                    nc.vector.tensor_copy(rsb[:], ps3b[0:6, :])
                    # -- scatter into batch-major rawbm [128, 6F]
                    for c in range(6):
                        nc.gpsimd.dma_start(
                            rawbm[k * PPC:(k + 1) * PPC, c * F:(c + 1) * F],
                            rsb[c:c + 1, :].rearrange("o (a f) -> o a f", f=F))

                # ---- coupling for slice s (batch-major [128, 6F] tiles)
                tt = p_cpl.tile([128, 6 * F], f32, tag="T")
                for c in range(6):
                    nc.scalar.activation(blk(tt, c), blk(rawbm, c), AF.Tanh,
                                         bias=t_cst[:, 1 + c:2 + c], scale=0.5)
                A = p_cpl.tile([128, 6 * F], f32, tag="A")
                nc.vector.tensor_scalar(A[:], tt[:], 0.5, 0.5, OP.mult, OP.add)
                for it in range(CF_ITERS):
                    D = p_cpl.tile([128, 6 * F], f32, tag="D")
                    for c in range(6):
                        nc.vector.tensor_scalar_mul(blk(D, c), blk(A, c),
                                                    float(DECAY[c]))
                    SN = p_cpl.tile([128, 6 * F], f32, tag="SN")
                    nc.scalar.activation(SN[:], D[:], AF.Sin)
                    CS = p_cpl.tile([128, 6 * F], f32, tag="CS")
                    nc.scalar.activation(CS[:], D[:], AF.Sin, bias=t_cst[:, 0:1])
                    P = p_cpl.tile([128, 6 * F], f32, tag="P")
                    Q = p_cpl.tile([128, 6 * F], f32, tag="Q")
                    for (i, j) in init_pairs:
                        nc.vector.tensor_scalar(pair_out(P, i, j),
                                                pair_src(SN, i, j),
                                                float(KC[i][j]), None, OP.mult)
                        nc.vector.tensor_scalar(pair_out(Q, i, j),
                                                pair_src(CS, i, j),
                                                float(KC[i][j]), None, OP.mult)
                    for (i, j) in rest_pairs:
                        nc.vector.scalar_tensor_tensor(
                            pair_out(P, i, j), pair_src(SN, i, j),
                            float(KC[i][j]), pair_out(P, i, j),
                            OP.mult, OP.add)
                        nc.vector.scalar_tensor_tensor(
                            pair_out(Q, i, j), pair_src(CS, i, j),
                            float(KC[i][j]), pair_out(Q, i, j),
                            OP.mult, OP.add)
                    U1 = p_cpl.tile([128, 6 * F], f32, tag="U1")
                    nc.vector.tensor_tensor(U1[:], CS[:], P[:], OP.mult)
                    U2 = p_cpl.tile([128, 6 * F], f32, tag="U2")
                    nc.vector.tensor_tensor(U2[:], SN[:], Q[:], OP.mult)
                    DD = p_cpl.tile([128, 6 * F], f32, tag="DD")
                    nc.vector.tensor_tensor(DD[:], U1[:], U2[:], OP.subtract)
                    V = p_cpl.tile([128, 6 * F], f32, tag="V")
                    nc.vector.tensor_tensor(V[:], D[:], DD[:], OP.add)
                    A = p_cpl.tile([128, 6 * F], f32, tag="A")
                    nc.vector.tensor_scalar(A[:], V[:], 0.0, 1.0, OP.max, OP.min)

                # ---- outputs: interleave [p, c*F+f] -> [p, f*6+c], then DMA
                OA = p_out.tile([128, 6 * F], f32, tag="oa")
                nc.vector.tensor_copy(
                    OA[:].rearrange("p (f c) -> p c f", c=6),
                    A[:].rearrange("p (c f) -> p c f", f=F))
                nc.gpsimd.dma_start(
                    act_o[s * slice_rows:(s + 1) * slice_rows, :]
                    .rearrange("(p x) c -> p (x c)", p=128),
                    OA[:])
                OR = p_out.tile([128, 6 * F], f32, tag="orr")
                nc.vector.tensor_copy(
                    OR[:].rearrange("p (f c) -> p c f", c=6),
                    rawbm[:].rearrange("p (c f) -> p c f", f=F))
                nc.gpsimd.dma_start(
                    raw_o[s * slice_rows:(s + 1) * slice_rows, :]
                    .rearrange("(p x) c -> p (x c)", p=128),
                    OR[:])
    return nc, emit


def prep_weights(W1, b1, W2, b2, W3, b3, W4, b4):
    """Host-side weight layout preparation (all fp32, C-contiguous)."""
    d = {}
    d["w1t"] = np.ascontiguousarray(W1.transpose(2, 0, 1).reshape(RES_DIM, 6 * 128))
    d["b1t"] = np.ascontiguousarray(b1.T)                      # [128, 6]
    d["w2t"] = np.ascontiguousarray(W2.transpose(2, 0, 1).reshape(128, 6 * 64))
    b2p = np.zeros((128, 3), np.float32)
    for p in range(3):
        b2p[0:64, p] = b2[2 * p]
        b2p[64:128, p] = b2[2 * p + 1]
    d["b2p"] = b2p
    w3t_h = W3.transpose(2, 0, 1).reshape(64, 6 * 32)
    d["w3t"] = np.ascontiguousarray(np.concatenate([w3t_h, w3t_h], axis=0))
    b3a = np.zeros((128, 1), np.float32)
    for c in range(4):
        b3a[32 * c:32 * (c + 1), 0] = b3[c]
    d["b3a"] = b3a
    b3b = np.zeros((128, 1), np.float32)
    b3b[64:96, 0] = b3[5]
    b3b[96:128, 0] = b3[4]
    d["b3b"] = b3b
    w4f = np.zeros((128, 12), np.float32)
    for c in range(4):
        w4f[32 * c:32 * (c + 1), c] = W4[c, 0, :]
    w4f[64:96, 6 + 5] = W4[5, 0, :]
    w4f[96:128, 6 + 4] = W4[4, 0, :]
    d["w4f"] = w4f
    cstv = np.zeros((128, 8), np.float32)
    cstv[:, 0] = HALF_PI
    for c in range(6):
        cstv[:, 1 + c] = 0.5 * b4[c, 0]
    d["cst"] = cstv
    d["_b4"] = np.ascontiguousarray(b4[:, 0])                  # host-only
    return d


def build_program(R=R_CORE, slice_rows=8192):
    """Build + bacc-compile the program (cached)."""
    key = (R, slice_rows)
    if key in _BUILD_CACHE:
        return _BUILD_CACHE[key]
    nc, emit = _build(R, slice_rows)
    emit()
    nc.compile()
    _BUILD_CACHE[key] = nc
    return nc


def kernel(res, W1, b1, W2, b2, W3, b3, W4, b4, coupling):
    """Full-input entry point: shards res over 8 cores, runs the SPMD
    kernel, gathers and returns (act, raw) like the reference."""
    from concourse.bass_utils import run_bass_kernel_spmd

    res = np.ascontiguousarray(np.asarray(res, np.float32))
    W1 = np.asarray(W1, np.float32); b1 = np.asarray(b1, np.float32)
    W2 = np.asarray(W2, np.float32); b2 = np.asarray(b2, np.float32)
    W3 = np.asarray(W3, np.float32); b3 = np.asarray(b3, np.float32)
    W4 = np.asarray(W4, np.float32); b4 = np.asarray(b4, np.float32)

    wd = prep_weights(W1, b1, W2, b2, W3, b3, W4, b4)
    b4vec = wd.pop("_b4")
    nc = build_program(R_CORE, 8192)

    in_maps = []
    for i in range(N_CORES):
        m = dict(wd)
        m["res"] = np.ascontiguousarray(res[i * R_CORE:(i + 1) * R_CORE])
        in_maps.append(m)
    out = run_bass_kernel_spmd(nc, in_maps, list(range(N_CORES)))
    act = np.concatenate([out.results[i]["act_o"] for i in range(N_CORES)], axis=0)
    raw = np.concatenate([out.results[i]["raw_o"] for i in range(N_CORES)], axis=0)
    raw = raw + b4vec[None, :]
    return act.astype(np.float32), raw.astype(np.float32)


# revision 11
# speedup vs baseline: 1.4447x; 1.4447x over previous
"""Trainium2 Bass kernel for nn_Chambers: 6 per-chamber MLPs over a shared
reservoir input, followed by 5 coupled-chamber fixed-point iterations.

Data-parallel over 8 NeuronCores: each core processes B/8 = 32768 rows.

Per-core pipeline (feature-major MLP, batch-major coupling):
  - res [R,100] loaded in [128,100] row tiles, transposed on the PE
    (is_transpose matmul vs identity) into resT [100, N] in PSUM, copied
    to SBUF by the DVE.
  - L1 (K=100,M=128) per chamber; silu+bias fused into one ACT op
    reading PSUM (bias is a per-partition AP).
  - L2 (K=128,M=64): two chambers run concurrently via column tiling
    (tile_position (0,0)/(0,64)), one [128,N] PSUM tile -> one silu op.
  - L3 (K=64,M=32): six chambers concurrent via row+column tiling.
  - L4: per-chamber dot products are K-stacked into two accumulating
    matmuls (block-column lhsT) producing raw = z4 as a [6,N] PSUM tile.
  - raw is DMA-scattered into batch-major [128, 6*F] tiles; sigmoid is
    computed as 0.5+0.5*tanh(0.5*x+0.5*b4) so every ACT function used
    (Silu/Tanh/Sin) lives in the single `silu_and_others` table set.
  - 5 coupling iterations run on the DVE using sin/cos expansion:
      delta_i = K*( cos a_i * (C sin a)_i - sin a_i * (C cos a)_i )
    with the 6x6 symmetric matvec done as 15 paired
    scalar_tensor_tensor AXPYs ([2,F] strided block-pair APs).
  - Coupling runs per batch slice so it overlaps the next slice's MLP.

Outputs: act (post-coupling) and raw (z4; b4 added on host).
"""

import numpy as np

# ---- problem constants (fixed by the task; kernel.py must be self-contained)
B = 262144
RES_DIM = 100
NCH = 6
CF_ITERS = 5
CF_K = 0.02
DECAY = np.array([0.9, 0.93, 0.85, 0.97, 0.88, 0.94], dtype=np.float32)
COUPLING = np.array([
    [0.0, -0.3, 0.6, 0.4, -0.2, 0.3],
    [-0.3, 0.0, -0.5, -0.7, 0.6, 0.4],
    [0.6, -0.5, 0.0, 0.3, -0.3, 0.2],
    [0.4, -0.7, 0.3, 0.0, -0.4, 0.5],
    [-0.2, 0.6, -0.3, -0.4, 0.0, 0.3],
    [0.3, 0.4, 0.2, 0.5, 0.3, 0.0]], dtype=np.float32)
N_CORES = 8
R_CORE = B // N_CORES          # 32768 rows per core
CHUNK = 1024                   # rows per MLP chunk
HALF_PI = float(np.pi / 2.0)

_BUILD_CACHE = {}


def _build(R, slice_rows):
    """Emit + compile the per-core SPMD program for R rows, coupling in
    slices of slice_rows. Returns the compiled Bacc object."""
    from contextlib import ExitStack
    import concourse.bass as bass
    import concourse.mybir as mybir
    from concourse import bacc, tile, masks

    f32 = mybir.dt.float32
    AF = mybir.ActivationFunctionType
    OP = mybir.AluOpType

    assert R % slice_rows == 0 and slice_rows % CHUNK == 0
    n_slices = R // slice_rows
    chunks_per_slice = slice_rows // CHUNK
    F = slice_rows // 128          # free cols per chamber in coupling tiles
    PPC = CHUNK // F               # partitions covered by one chunk's scatter
    KC = (CF_K * COUPLING).astype(np.float64)
    # (i,j) pairs: 3 "init" pairs covering each block once, 12 accumulating
    init_pairs = [(0, 1), (2, 3), (4, 5)]
    rest_pairs = [(i, j) for i in range(6) for j in range(i + 1, 6)
                  if (i, j) not in init_pairs]

    nc = bacc.Bacc("TRN2", target_bir_lowering=False, debug=False,
                   num_devices=N_CORES)
    bf16 = mybir.dt.bfloat16
    # res zero-padded to 128 cols, bf16, so the 2-byte xbar DMA-transpose
    # can load it DRAM->SBUF already transposed (128x128 tiles)
    res = nc.dram_tensor("res_pad", [R, 128], bf16, kind="ExternalInput").ap()
    w1t = nc.dram_tensor("w1t", [RES_DIM, 6 * 128], bf16, kind="ExternalInput").ap()
    b1t = nc.dram_tensor("b1t", [128, 6], f32, kind="ExternalInput").ap()
    w2t = nc.dram_tensor("w2t", [128, 6 * 64], bf16, kind="ExternalInput").ap()
    b2p = nc.dram_tensor("b2p", [128, 3], f32, kind="ExternalInput").ap()
    # w3t holds W3^T twice (rows 0-63 and 64-127): row-tiled matmuls need
    # the stationary operand at the same base partition as the moving one
    w3t = nc.dram_tensor("w3t", [128, 6 * 32], bf16, kind="ExternalInput").ap()
    b3a = nc.dram_tensor("b3a", [128, 1], f32, kind="ExternalInput").ap()
    b3b = nc.dram_tensor("b3b", [128, 1], f32, kind="ExternalInput").ap()
    w4f = nc.dram_tensor("w4f", [128, 12], bf16, kind="ExternalInput").ap()
    # cst col 0 = pi/2 (cos bias); cols 1..6 = 0.5*b4[c] (tanh biases)
    cst = nc.dram_tensor("cst", [128, 8], f32, kind="ExternalInput").ap()
    act_o = nc.dram_tensor("act_o", [R, 6], f32, kind="ExternalOutput").ap()
    raw_o = nc.dram_tensor("raw_o", [R, 6], f32, kind="ExternalOutput").ap()

    def emit():
        with tile.TileContext(nc) as tc, ExitStack() as ctx:
            wp = ctx.enter_context(tc.tile_pool(name="w", bufs=1))
            t_w1t = wp.tile([RES_DIM, 6 * 128], bf16, tag="w1t")
            nc.gpsimd.dma_start(t_w1t[:], w1t)
            t_b1t = wp.tile([128, 6], f32, tag="b1t")
            nc.gpsimd.dma_start(t_b1t[:], b1t)
            t_w2t = wp.tile([128, 6 * 64], bf16, tag="w2t")
            nc.gpsimd.dma_start(t_w2t[:], w2t)
            t_b2p = wp.tile([128, 3], f32, tag="b2p")
            nc.gpsimd.dma_start(t_b2p[:], b2p)
            t_w3t = wp.tile([128, 6 * 32], bf16, tag="w3t")
            nc.gpsimd.dma_start(t_w3t[:], w3t)
            t_b3a = wp.tile([128, 1], f32, tag="b3a")
            nc.gpsimd.dma_start(t_b3a[:], b3a)
            t_b3b = wp.tile([128, 1], f32, tag="b3b")
            nc.gpsimd.dma_start(t_b3b[:], b3b)
            t_w4f = wp.tile([128, 12], bf16, tag="w4f")
            nc.gpsimd.dma_start(t_w4f[:], w4f)
            t_cst = wp.tile([128, 8], f32, tag="cst")
            nc.gpsimd.dma_start(t_cst[:], cst)
            p_rT = ctx.enter_context(tc.tile_pool(name="rT", bufs=3))
            p_mm = ctx.enter_context(tc.tile_pool(name="pmm", bufs=3, space="PSUM"))
            p_p4 = ctx.enter_context(tc.tile_pool(name="p4", bufs=1, space="PSUM"))
            p_h1 = ctx.enter_context(tc.tile_pool(name="h1", bufs=3))
            p_h2 = ctx.enter_context(tc.tile_pool(name="h2", bufs=4))
            p_h3 = ctx.enter_context(tc.tile_pool(name="h3", bufs=2))
            p_rsb = ctx.enter_context(tc.tile_pool(name="rsb", bufs=2))
            p_bm = ctx.enter_context(tc.tile_pool(name="bm", bufs=2))
            p_cpl = ctx.enter_context(tc.tile_pool(name="cpl", bufs=2))
            p_out = ctx.enter_context(tc.tile_pool(name="out", bufs=2))

            def blk(ap_t, c):
                return ap_t[:, c * F:(c + 1) * F]

            def pair_out(ap_t, i, j):
                x3 = ap_t[:].rearrange("p (c f) -> p c f", f=F)
                return x3[:, i:j + 1:(j - i), :]

            def pair_src(ap_t, i, j):
                # blocks [j, i] (swapped) via negative step
                x3 = ap_t[:].rearrange("p (c f) -> p c f", f=F)
                if i == 0:
                    return x3[:, j::-(j - i), :][:, 0:2, :]
                return x3[:, j:i - 1:-(j - i), :]

            for s in range(n_slices):
                rawbm = p_bm.tile([128, 6 * F], f32, tag="rawbm")
                for k in range(chunks_per_slice):
                    g = s * chunks_per_slice + k       # global chunk id
                    # -- load res transposed via the 2-byte xbar DMA
                    # (rows 100..127 hold transposed zero padding)
                    rTf = p_rT.tile([128, CHUNK], bf16, tag="rT")
                    rT = rTf[0:RES_DIM, :]
                    for j in range(8):
                        nc.sync.dma_start(
                            rTf[:, j * 128:(j + 1) * 128],
                            res[g * CHUNK + j * 128:g * CHUNK + (j + 1) * 128, :],
                            transpose=True)

                    # -- L1 + L2 (paired)
                    h1s = {}
                    h2s = {}
                    for c in range(6):
                        ps1 = p_mm.tile([128, CHUNK], f32, tag="mm")
                        for h in range(2):
                            nc.tensor.matmul(
                                ps1[:, h * 512:(h + 1) * 512],
                                t_w1t[:, c * 128:(c + 1) * 128],
                                rT[:, h * 512:(h + 1) * 512])
                        h1 = p_h1.tile([128, CHUNK], bf16, tag="h1")
                        nc.scalar.activation(h1[:], ps1[:], AF.Silu,
                                             bias=t_b1t[:, c:c + 1])
                        h1s[c] = h1
                        if c % 2 == 1:
                            p = c // 2
                            ps2 = p_mm.tile([128, CHUNK], f32, tag="mm")
                            for h in range(2):
                                nc.tensor.matmul(
                                    ps2[0:64, h * 512:(h + 1) * 512],
                                    t_w2t[:, (2 * p) * 64:(2 * p + 1) * 64],
                                    h1s[2 * p][:, h * 512:(h + 1) * 512],
                                    tile_position=(0, 0))
                                nc.tensor.matmul(
                                    ps2[64:128, h * 512:(h + 1) * 512],
                                    t_w2t[:, (2 * p + 1) * 64:(2 * p + 2) * 64],
                                    h1s[2 * p + 1][:, h * 512:(h + 1) * 512],
                                    tile_position=(0, 64))
                            h2 = p_h2.tile([128, CHUNK], bf16, tag="h2")
                            nc.scalar.activation(h2[:], ps2[:], AF.Silu,
                                                 bias=t_b2p[:, p:p + 1])
                            h2s[p] = h2

                    # -- L3: chambers 0-3 -> ps3a rows 0..127; c5 -> ps3b rows
                    # 64..95, c4 -> ps3b rows 96..127 (row base for K=64 must
                    # be 0/64; all six (row,col) subarray sets stay distinct)
                    ps3a = p_mm.tile([128, CHUNK], f32, tag="mm")
                    ps3b = p_p4.tile([128, CHUNK], f32, tag="p4")
                    # (chamber, h2 pair, rows(0=upper 64,1=lower), out tile, out row base)
                    l3 = [(0, 0, 0, ps3a, 0), (1, 0, 1, ps3a, 32),
                          (2, 1, 0, ps3a, 64), (3, 1, 1, ps3a, 96),
                          (4, 2, 0, ps3b, 96), (5, 2, 1, ps3b, 64)]
                    for (c, p, half, pst, rb) in l3:
                        for h in range(2):
                            nc.tensor.matmul(
                                pst[rb:rb + 32, h * 512:(h + 1) * 512],
                                t_w3t[64 * half:64 * half + 64, c * 32:(c + 1) * 32],
                                h2s[p][64 * half:64 * half + 64,
                                       h * 512:(h + 1) * 512],
                                tile_position=(64 * half, rb))
                    h3a = p_h3.tile([128, CHUNK], bf16, tag="h3a")
                    nc.scalar.activation(h3a[:], ps3a[:], AF.Silu, bias=t_b3a[:])
                    h3b = p_h3.tile([128, CHUNK], bf16, tag="h3b")
                    nc.scalar.activation(h3b[64:128, :], ps3b[64:128, :], AF.Silu,
                                         bias=t_b3b[64:128, :])

                    # -- L4: raw[0:6] = w4f.T @ h3 (K-stacked accumulation)
                    for h in range(2):
                        nc.tensor.matmul(
                            ps3b[0:6, h * 512:(h + 1) * 512],
                            t_w4f[:, 0:6],
                            h3a[:, h * 512:(h + 1) * 512],
                            start=True, stop=False, tile_position=(0, 0))
                        nc.tensor.matmul(
                            ps3b[0:6, h * 512:(h + 1) * 512],
                            t_w4f[64:128, 6:12],
                            h3b[64:128, h * 512:(h + 1) * 512],
                            start=False, stop=True, tile_position=(64, 0))
                    rsb = p_rsb.tile([6, CHUNK], f32, tag="rsb")
                    nc.vector.tensor_copy(rsb[:], ps3b[0:6, :])
                    # -- scatter into batch-major rawbm [128, 6F]
                    for c in range(6):
                        nc.gpsimd.dma_start(
                            rawbm[k * PPC:(k + 1) * PPC, c * F:(c + 1) * F],
                            rsb[c:c + 1, :].rearrange("o (a f) -> o a f", f=F))

                # ---- coupling for slice s (batch-major [128, 6F] tiles)
                tt = p_cpl.tile([128, 6 * F], f32, tag="T")
                for c in range(6):
                    nc.scalar.activation(blk(tt, c), blk(rawbm, c), AF.Tanh,
                                         bias=t_cst[:, 1 + c:2 + c], scale=0.5)
                A = p_cpl.tile([128, 6 * F], f32, tag="A")
                nc.vector.tensor_scalar(A[:], tt[:], 0.5, 0.5, OP.mult, OP.add)
                for it in range(CF_ITERS):
                    D = p_cpl.tile([128, 6 * F], f32, tag="D")
                    for c in range(6):
                        nc.vector.tensor_scalar_mul(blk(D, c), blk(A, c),
                                                    float(DECAY[c]))
                    SN = p_cpl.tile([128, 6 * F], f32, tag="SN")
                    nc.scalar.activation(SN[:], D[:], AF.Sin)
                    CS = p_cpl.tile([128, 6 * F], f32, tag="CS")
                    nc.scalar.activation(CS[:], D[:], AF.Sin, bias=t_cst[:, 0:1])
                    P = p_cpl.tile([128, 6 * F], f32, tag="P")
                    Q = p_cpl.tile([128, 6 * F], f32, tag="Q")
                    for (i, j) in init_pairs:
                        nc.vector.tensor_scalar(pair_out(P, i, j),
                                                pair_src(SN, i, j),
                                                float(KC[i][j]), None, OP.mult)
                        nc.vector.tensor_scalar(pair_out(Q, i, j),
                                                pair_src(CS, i, j),
                                                float(KC[i][j]), None, OP.mult)
                    for (i, j) in rest_pairs:
                        nc.vector.scalar_tensor_tensor(
                            pair_out(P, i, j), pair_src(SN, i, j),
                            float(KC[i][j]), pair_out(P, i, j),
                            OP.mult, OP.add)
                        nc.vector.scalar_tensor_tensor(
                            pair_out(Q, i, j), pair_src(CS, i, j),
                            float(KC[i][j]), pair_out(Q, i, j),
                            OP.mult, OP.add)
                    U1 = p_cpl.tile([128, 6 * F], f32, tag="U1")
                    nc.vector.tensor_tensor(U1[:], CS[:], P[:], OP.mult)
                    U2 = p_cpl.tile([128, 6 * F], f32, tag="U2")
                    nc.vector.tensor_tensor(U2[:], SN[:], Q[:], OP.mult)
                    DD = p_cpl.tile([128, 6 * F], f32, tag="DD")
                    nc.vector.tensor_tensor(DD[:], U1[:], U2[:], OP.subtract)
                    V = p_cpl.tile([128, 6 * F], f32, tag="V")
                    nc.vector.tensor_tensor(V[:], D[:], DD[:], OP.add)
                    A = p_cpl.tile([128, 6 * F], f32, tag="A")
                    nc.vector.tensor_scalar(A[:], V[:], 0.0, 1.0, OP.max, OP.min)

                # ---- outputs: interleave [p, c*F+f] -> [p, f*6+c], then DMA
                OA = p_out.tile([128, 6 * F], f32, tag="oa")
                nc.vector.tensor_copy(
                    OA[:].rearrange("p (f c) -> p c f", c=6),
                    A[:].rearrange("p (c f) -> p c f", f=F))
                nc.gpsimd.dma_start(
                    act_o[s * slice_rows:(s + 1) * slice_rows, :]
                    .rearrange("(p x) c -> p (x c)", p=128),
                    OA[:])
                OR = p_out.tile([128, 6 * F], f32, tag="orr")
                nc.vector.tensor_copy(
                    OR[:].rearrange("p (f c) -> p c f", c=6),
                    rawbm[:].rearrange("p (c f) -> p c f", f=F))
                nc.gpsimd.dma_start(
                    raw_o[s * slice_rows:(s + 1) * slice_rows, :]
                    .rearrange("(p x) c -> p (x c)", p=128),
                    OR[:])
    return nc, emit


def prep_weights(W1, b1, W2, b2, W3, b3, W4, b4):
    """Host-side weight layout preparation."""
    import ml_dtypes
    bf16 = ml_dtypes.bfloat16
    d = {}
    d["w1t"] = np.ascontiguousarray(
        W1.transpose(2, 0, 1).reshape(RES_DIM, 6 * 128)).astype(bf16)
    d["b1t"] = np.ascontiguousarray(b1.T)                      # [128, 6]
    d["w2t"] = np.ascontiguousarray(
        W2.transpose(2, 0, 1).reshape(128, 6 * 64)).astype(bf16)
    b2p = np.zeros((128, 3), np.float32)
    for p in range(3):
        b2p[0:64, p] = b2[2 * p]
        b2p[64:128, p] = b2[2 * p + 1]
    d["b2p"] = b2p
    w3t_h = W3.transpose(2, 0, 1).reshape(64, 6 * 32)
    d["w3t"] = np.ascontiguousarray(
        np.concatenate([w3t_h, w3t_h], axis=0)).astype(bf16)
    b3a = np.zeros((128, 1), np.float32)
    for c in range(4):
        b3a[32 * c:32 * (c + 1), 0] = b3[c]
    d["b3a"] = b3a
    b3b = np.zeros((128, 1), np.float32)
    b3b[64:96, 0] = b3[5]
    b3b[96:128, 0] = b3[4]
    d["b3b"] = b3b
    w4f = np.zeros((128, 12), np.float32)
    for c in range(4):
        w4f[32 * c:32 * (c + 1), c] = W4[c, 0, :]
    w4f[64:96, 6 + 5] = W4[5, 0, :]
    w4f[96:128, 6 + 4] = W4[4, 0, :]
    d["w4f"] = w4f.astype(bf16)
    cstv = np.zeros((128, 8), np.float32)
    cstv[:, 0] = HALF_PI
    for c in range(6):
        cstv[:, 1 + c] = 0.5 * b4[c, 0]
    d["cst"] = cstv
    d["_b4"] = np.ascontiguousarray(b4[:, 0])                  # host-only
    return d


def build_program(R=R_CORE, slice_rows=8192):
    """Build + bacc-compile the program (cached)."""
    key = (R, slice_rows)
    if key in _BUILD_CACHE:
        return _BUILD_CACHE[key]
    nc, emit = _build(R, slice_rows)
    emit()
    nc.compile()
    _BUILD_CACHE[key] = nc
    return nc


def kernel(res, W1, b1, W2, b2, W3, b3, W4, b4, coupling):
    """Full-input entry point: shards res over 8 cores, runs the SPMD
    kernel, gathers and returns (act, raw) like the reference."""
    from concourse.bass_utils import run_bass_kernel_spmd

    res = np.ascontiguousarray(np.asarray(res, np.float32))
    W1 = np.asarray(W1, np.float32); b1 = np.asarray(b1, np.float32)
    W2 = np.asarray(W2, np.float32); b2 = np.asarray(b2, np.float32)
    W3 = np.asarray(W3, np.float32); b3 = np.asarray(b3, np.float32)
    W4 = np.asarray(W4, np.float32); b4 = np.asarray(b4, np.float32)

    wd = prep_weights(W1, b1, W2, b2, W3, b3, W4, b4)
    b4vec = wd.pop("_b4")
    nc = build_program(R_CORE, 8192)

    import ml_dtypes
    res_pad = np.zeros((B, 128), ml_dtypes.bfloat16)
    res_pad[:, :RES_DIM] = res.astype(ml_dtypes.bfloat16)
    in_maps = []
    for i in range(N_CORES):
        m = dict(wd)
        m["res_pad"] = np.ascontiguousarray(res_pad[i * R_CORE:(i + 1) * R_CORE])
        in_maps.append(m)
    out = run_bass_kernel_spmd(nc, in_maps, list(range(N_CORES)))
    act = np.concatenate([out.results[i]["act_o"] for i in range(N_CORES)], axis=0)
    raw = np.concatenate([out.results[i]["raw_o"] for i in range(N_CORES)], axis=0)
    raw = raw + b4vec[None, :]
    return act.astype(np.float32), raw.astype(np.float32)


# revision 12
# speedup vs baseline: 1.6746x; 1.1592x over previous
"""Trainium2 Bass kernel for nn_Chambers: 6 per-chamber MLPs over a shared
reservoir input, followed by 5 coupled-chamber fixed-point iterations.

Data-parallel over 8 NeuronCores: each core processes B/8 = 32768 rows.

Per-core pipeline (feature-major MLP, batch-major coupling):
  - res [R,100] loaded in [128,100] row tiles, transposed on the PE
    (is_transpose matmul vs identity) into resT [100, N] in PSUM, copied
    to SBUF by the DVE.
  - L1 (K=100,M=128) per chamber; silu+bias fused into one ACT op
    reading PSUM (bias is a per-partition AP).
  - L2 (K=128,M=64): two chambers run concurrently via column tiling
    (tile_position (0,0)/(0,64)), one [128,N] PSUM tile -> one silu op.
  - L3 (K=64,M=32): six chambers concurrent via row+column tiling.
  - L4: per-chamber dot products are K-stacked into two accumulating
    matmuls (block-column lhsT) producing raw = z4 as a [6,N] PSUM tile.
  - raw is DMA-scattered into batch-major [128, 6*F] tiles; sigmoid is
    computed as 0.5+0.5*tanh(0.5*x+0.5*b4) so every ACT function used
    (Silu/Tanh/Sin) lives in the single `silu_and_others` table set.
  - 5 coupling iterations run on the DVE using sin/cos expansion:
      delta_i = K*( cos a_i * (C sin a)_i - sin a_i * (C cos a)_i )
    with the 6x6 symmetric matvec done as 15 paired
    scalar_tensor_tensor AXPYs ([2,F] strided block-pair APs).
  - Coupling runs per batch slice so it overlaps the next slice's MLP.

Outputs: act (post-coupling) and raw (z4; b4 added on host).
"""

import numpy as np

# ---- problem constants (fixed by the task; kernel.py must be self-contained)
B = 262144
RES_DIM = 100
NCH = 6
CF_ITERS = 5
CF_K = 0.02
DECAY = np.array([0.9, 0.93, 0.85, 0.97, 0.88, 0.94], dtype=np.float32)
COUPLING = np.array([
    [0.0, -0.3, 0.6, 0.4, -0.2, 0.3],
    [-0.3, 0.0, -0.5, -0.7, 0.6, 0.4],
    [0.6, -0.5, 0.0, 0.3, -0.3, 0.2],
    [0.4, -0.7, 0.3, 0.0, -0.4, 0.5],
    [-0.2, 0.6, -0.3, -0.4, 0.0, 0.3],
    [0.3, 0.4, 0.2, 0.5, 0.3, 0.0]], dtype=np.float32)
N_CORES = 8
R_CORE = B // N_CORES          # 32768 rows per core
CHUNK = 1024                   # rows per MLP chunk
HALF_PI = float(np.pi / 2.0)

_BUILD_CACHE = {}


def _build(R, slice_rows):
    """Emit + compile the per-core SPMD program for R rows, coupling in
    slices of slice_rows. Returns the compiled Bacc object."""
    from contextlib import ExitStack
    import concourse.bass as bass
    import concourse.mybir as mybir
    from concourse import bacc, tile, masks

    f32 = mybir.dt.float32
    AF = mybir.ActivationFunctionType
    OP = mybir.AluOpType

    assert R % slice_rows == 0 and slice_rows % CHUNK == 0
    n_slices = R // slice_rows
    chunks_per_slice = slice_rows // CHUNK
    F = slice_rows // 128          # free cols per chamber in coupling tiles
    PPC = CHUNK // F               # partitions covered by one chunk's scatter
    KC = (CF_K * COUPLING).astype(np.float64)
    # (i,j) pairs: 3 "init" pairs covering each block once, 12 accumulating
    init_pairs = [(0, 1), (2, 3), (4, 5)]
    rest_pairs = [(i, j) for i in range(6) for j in range(i + 1, 6)
                  if (i, j) not in init_pairs]

    nc = bacc.Bacc("TRN2", target_bir_lowering=False, debug=False,
                   num_devices=N_CORES)
    bf16 = mybir.dt.bfloat16
    # res zero-padded to 128 cols, bf16, so the 2-byte xbar DMA-transpose
    # can load it DRAM->SBUF already transposed (128x128 tiles)
    res = nc.dram_tensor("res_pad", [R, 128], bf16, kind="ExternalInput").ap()
    w1t = nc.dram_tensor("w1t", [RES_DIM, 6 * 128], bf16, kind="ExternalInput").ap()
    b1t = nc.dram_tensor("b1t", [128, 6], f32, kind="ExternalInput").ap()
    w2t = nc.dram_tensor("w2t", [128, 6 * 64], bf16, kind="ExternalInput").ap()
    b2p = nc.dram_tensor("b2p", [128, 3], f32, kind="ExternalInput").ap()
    # w3t holds W3^T twice (rows 0-63 and 64-127): row-tiled matmuls need
    # the stationary operand at the same base partition as the moving one
    w3t = nc.dram_tensor("w3t", [128, 6 * 32], bf16, kind="ExternalInput").ap()
    b3a = nc.dram_tensor("b3a", [128, 1], f32, kind="ExternalInput").ap()
    b3b = nc.dram_tensor("b3b", [128, 1], f32, kind="ExternalInput").ap()
    w4f = nc.dram_tensor("w4f", [128, 12], bf16, kind="ExternalInput").ap()
    # cst col 0 = pi/2 (cos bias); cols 1..6 = 0.5*b4[c] (tanh biases)
    cst = nc.dram_tensor("cst", [128, 8], f32, kind="ExternalInput").ap()
    act_o = nc.dram_tensor("act_o", [R, 6], f32, kind="ExternalOutput").ap()
    raw_o = nc.dram_tensor("raw_o", [R, 6], f32, kind="ExternalOutput").ap()

    def emit():
        with tile.TileContext(nc) as tc, ExitStack() as ctx:
            wp = ctx.enter_context(tc.tile_pool(name="w", bufs=1))
            t_w1t = wp.tile([RES_DIM, 6 * 128], bf16, tag="w1t")
            nc.gpsimd.dma_start(t_w1t[:], w1t)
            t_b1t = wp.tile([128, 6], f32, tag="b1t")
            nc.gpsimd.dma_start(t_b1t[:], b1t)
            t_w2t = wp.tile([128, 6 * 64], bf16, tag="w2t")
            nc.gpsimd.dma_start(t_w2t[:], w2t)
            t_b2p = wp.tile([128, 3], f32, tag="b2p")
            nc.gpsimd.dma_start(t_b2p[:], b2p)
            t_w3t = wp.tile([128, 6 * 32], bf16, tag="w3t")
            nc.gpsimd.dma_start(t_w3t[:], w3t)
            t_b3a = wp.tile([128, 1], f32, tag="b3a")
            nc.gpsimd.dma_start(t_b3a[:], b3a)
            t_b3b = wp.tile([128, 1], f32, tag="b3b")
            nc.gpsimd.dma_start(t_b3b[:], b3b)
            t_w4f = wp.tile([128, 12], bf16, tag="w4f")
            nc.gpsimd.dma_start(t_w4f[:], w4f)
            t_cst = wp.tile([128, 8], f32, tag="cst")
            nc.gpsimd.dma_start(t_cst[:], cst)
            p_rT = ctx.enter_context(tc.tile_pool(name="rT", bufs=3))
            p_mm = ctx.enter_context(tc.tile_pool(name="pmm", bufs=3, space="PSUM"))
            p_p4 = ctx.enter_context(tc.tile_pool(name="p4", bufs=1, space="PSUM"))
            p_h1 = ctx.enter_context(tc.tile_pool(name="h1", bufs=3))
            p_h2 = ctx.enter_context(tc.tile_pool(name="h2", bufs=4))
            p_h3 = ctx.enter_context(tc.tile_pool(name="h3", bufs=2))
            p_rsb = ctx.enter_context(tc.tile_pool(name="rsb", bufs=2))
            p_bm = ctx.enter_context(tc.tile_pool(name="bm", bufs=2))
            p_cpl = ctx.enter_context(tc.tile_pool(name="cpl", bufs=2))
            p_out = ctx.enter_context(tc.tile_pool(name="out", bufs=2))

            def blk(ap_t, c):
                return ap_t[:, c * F:(c + 1) * F]

            def pair_out(ap_t, i, j):
                x3 = ap_t[:].rearrange("p (c f) -> p c f", f=F)
                return x3[:, i:j + 1:(j - i), :]

            def pair_src(ap_t, i, j):
                # blocks [j, i] (swapped) via negative step
                x3 = ap_t[:].rearrange("p (c f) -> p c f", f=F)
                if i == 0:
                    return x3[:, j::-(j - i), :][:, 0:2, :]
                return x3[:, j:i - 1:-(j - i), :]

            for s in range(n_slices):
                rawbm = p_bm.tile([128, 6 * F], f32, tag="rawbm")
                for k in range(chunks_per_slice):
                    g = s * chunks_per_slice + k       # global chunk id
                    # -- load res transposed via the 2-byte xbar DMA
                    # (rows 100..127 hold transposed zero padding)
                    rTf = p_rT.tile([128, CHUNK], bf16, tag="rT")
                    rT = rTf[0:RES_DIM, :]
                    nc.sync.dma_start(
                        rTf[:],
                        res[g * CHUNK:(g + 1) * CHUNK, :],
                        transpose=True)

                    # -- L1 + L2 (paired)
                    h1s = {}
                    h2s = {}
                    for c in range(6):
                        ps1 = p_mm.tile([128, CHUNK], f32, tag="mm")
                        for h in range(2):
                            nc.tensor.matmul(
                                ps1[:, h * 512:(h + 1) * 512],
                                t_w1t[:, c * 128:(c + 1) * 128],
                                rT[:, h * 512:(h + 1) * 512])
                        h1 = p_h1.tile([128, CHUNK], bf16, tag="h1")
                        nc.scalar.activation(h1[:], ps1[:], AF.Silu,
                                             bias=t_b1t[:, c:c + 1])
                        h1s[c] = h1
                        if c % 2 == 1:
                            p = c // 2
                            ps2 = p_mm.tile([128, CHUNK], f32, tag="mm")
                            for h in range(2):
                                nc.tensor.matmul(
                                    ps2[0:64, h * 512:(h + 1) * 512],
                                    t_w2t[:, (2 * p) * 64:(2 * p + 1) * 64],
                                    h1s[2 * p][:, h * 512:(h + 1) * 512],
                                    tile_position=(0, 0))
                                nc.tensor.matmul(
                                    ps2[64:128, h * 512:(h + 1) * 512],
                                    t_w2t[:, (2 * p + 1) * 64:(2 * p + 2) * 64],
                                    h1s[2 * p + 1][:, h * 512:(h + 1) * 512],
                                    tile_position=(0, 64))
                            h2 = p_h2.tile([128, CHUNK], bf16, tag="h2")
                            nc.scalar.activation(h2[:], ps2[:], AF.Silu,
                                                 bias=t_b2p[:, p:p + 1])
                            h2s[p] = h2

                    # -- L3: chambers 0-3 -> ps3a rows 0..127; c5 -> ps3b rows
                    # 64..95, c4 -> ps3b rows 96..127 (row base for K=64 must
                    # be 0/64; all six (row,col) subarray sets stay distinct)
                    ps3a = p_mm.tile([128, CHUNK], f32, tag="mm")
                    ps3b = p_p4.tile([128, CHUNK], f32, tag="p4")
                    # (chamber, h2 pair, rows(0=upper 64,1=lower), out tile, out row base)
                    l3 = [(0, 0, 0, ps3a, 0), (1, 0, 1, ps3a, 32),
                          (2, 1, 0, ps3a, 64), (3, 1, 1, ps3a, 96),
                          (4, 2, 0, ps3b, 96), (5, 2, 1, ps3b, 64)]
                    for (c, p, half, pst, rb) in l3:
                        for h in range(2):
                            nc.tensor.matmul(
                                pst[rb:rb + 32, h * 512:(h + 1) * 512],
                                t_w3t[64 * half:64 * half + 64, c * 32:(c + 1) * 32],
                                h2s[p][64 * half:64 * half + 64,
                                       h * 512:(h + 1) * 512],
                                tile_position=(64 * half, rb))
                    h3a = p_h3.tile([128, CHUNK], bf16, tag="h3a")
                    nc.scalar.activation(h3a[:], ps3a[:], AF.Silu, bias=t_b3a[:])
                    h3b = p_h3.tile([128, CHUNK], bf16, tag="h3b")
                    nc.scalar.activation(h3b[64:128, :], ps3b[64:128, :], AF.Silu,
                                         bias=t_b3b[64:128, :])

                    # -- L4: raw[0:6] = w4f.T @ h3 (K-stacked accumulation)
                    for h in range(2):
                        nc.tensor.matmul(
                            ps3b[0:6, h * 512:(h + 1) * 512],
                            t_w4f[:, 0:6],
                            h3a[:, h * 512:(h + 1) * 512],
                            start=True, stop=False, tile_position=(0, 0))
                        nc.tensor.matmul(
                            ps3b[0:6, h * 512:(h + 1) * 512],
                            t_w4f[64:128, 6:12],
                            h3b[64:128, h * 512:(h + 1) * 512],
                            start=False, stop=True, tile_position=(64, 0))
                    rsb = p_rsb.tile([6, CHUNK], f32, tag="rsb")
                    nc.vector.tensor_copy(rsb[:], ps3b[0:6, :])
                    # -- scatter into batch-major rawbm [128, 6F]
                    for c in range(6):
                        nc.gpsimd.dma_start(
                            rawbm[k * PPC:(k + 1) * PPC, c * F:(c + 1) * F],
                            rsb[c:c + 1, :].rearrange("o (a f) -> o a f", f=F))

                # ---- coupling for slice s (batch-major [128, 6F] tiles)
                tt = p_cpl.tile([128, 6 * F], f32, tag="T")
                for c in range(6):
                    nc.scalar.activation(blk(tt, c), blk(rawbm, c), AF.Tanh,
                                         bias=t_cst[:, 1 + c:2 + c], scale=0.5)
                A = p_cpl.tile([128, 6 * F], f32, tag="A")
                nc.vector.tensor_scalar(A[:], tt[:], 0.5, 0.5, OP.mult, OP.add)
                for it in range(CF_ITERS):
                    D = p_cpl.tile([128, 6 * F], f32, tag="D")
                    for c in range(6):
                        nc.vector.tensor_scalar_mul(blk(D, c), blk(A, c),
                                                    float(DECAY[c]))
                    SN = p_cpl.tile([128, 6 * F], f32, tag="SN")
                    nc.scalar.activation(SN[:], D[:], AF.Sin)
                    CS = p_cpl.tile([128, 6 * F], f32, tag="CS")
                    nc.scalar.activation(CS[:], D[:], AF.Sin, bias=t_cst[:, 0:1])
                    P = p_cpl.tile([128, 6 * F], f32, tag="P")
                    Q = p_cpl.tile([128, 6 * F], f32, tag="Q")
                    for (i, j) in init_pairs:
                        nc.vector.tensor_scalar(pair_out(P, i, j),
                                                pair_src(SN, i, j),
                                                float(KC[i][j]), None, OP.mult)
                        nc.vector.tensor_scalar(pair_out(Q, i, j),
                                                pair_src(CS, i, j),
                                                float(KC[i][j]), None, OP.mult)
                    for (i, j) in rest_pairs:
                        nc.vector.scalar_tensor_tensor(
                            pair_out(P, i, j), pair_src(SN, i, j),
                            float(KC[i][j]), pair_out(P, i, j),
                            OP.mult, OP.add)
                        nc.vector.scalar_tensor_tensor(
                            pair_out(Q, i, j), pair_src(CS, i, j),
                            float(KC[i][j]), pair_out(Q, i, j),
                            OP.mult, OP.add)
                    U1 = p_cpl.tile([128, 6 * F], f32, tag="U1")
                    nc.vector.tensor_tensor(U1[:], CS[:], P[:], OP.mult)
                    U2 = p_cpl.tile([128, 6 * F], f32, tag="U2")
                    nc.vector.tensor_tensor(U2[:], SN[:], Q[:], OP.mult)
                    DD = p_cpl.tile([128, 6 * F], f32, tag="DD")
                    nc.vector.tensor_tensor(DD[:], U1[:], U2[:], OP.subtract)
                    V = p_cpl.tile([128, 6 * F], f32, tag="V")
                    nc.vector.tensor_tensor(V[:], D[:], DD[:], OP.add)
                    A = p_cpl.tile([128, 6 * F], f32, tag="A")
                    nc.vector.tensor_scalar(A[:], V[:], 0.0, 1.0, OP.max, OP.min)

                # ---- outputs: interleave [p, c*F+f] -> [p, f*6+c], then DMA
                OA = p_out.tile([128, 6 * F], f32, tag="oa")
                nc.vector.tensor_copy(
                    OA[:].rearrange("p (f c) -> p c f", c=6),
                    A[:].rearrange("p (c f) -> p c f", f=F))
                nc.gpsimd.dma_start(
                    act_o[s * slice_rows:(s + 1) * slice_rows, :]
                    .rearrange("(p x) c -> p (x c)", p=128),
                    OA[:])
                OR = p_out.tile([128, 6 * F], f32, tag="orr")
                nc.vector.tensor_copy(
                    OR[:].rearrange("p (f c) -> p c f", c=6),
                    rawbm[:].rearrange("p (c f) -> p c f", f=F))
                nc.gpsimd.dma_start(
                    raw_o[s * slice_rows:(s + 1) * slice_rows, :]
                    .rearrange("(p x) c -> p (x c)", p=128),
                    OR[:])
    return nc, emit


def prep_weights(W1, b1, W2, b2, W3, b3, W4, b4):
    """Host-side weight layout preparation."""
    import ml_dtypes
    bf16 = ml_dtypes.bfloat16
    d = {}
    d["w1t"] = np.ascontiguousarray(
        W1.transpose(2, 0, 1).reshape(RES_DIM, 6 * 128)).astype(bf16)
    d["b1t"] = np.ascontiguousarray(b1.T)                      # [128, 6]
    d["w2t"] = np.ascontiguousarray(
        W2.transpose(2, 0, 1).reshape(128, 6 * 64)).astype(bf16)
    b2p = np.zeros((128, 3), np.float32)
    for p in range(3):
        b2p[0:64, p] = b2[2 * p]
        b2p[64:128, p] = b2[2 * p + 1]
    d["b2p"] = b2p
    w3t_h = W3.transpose(2, 0, 1).reshape(64, 6 * 32)
    d["w3t"] = np.ascontiguousarray(
        np.concatenate([w3t_h, w3t_h], axis=0)).astype(bf16)
    b3a = np.zeros((128, 1), np.float32)
    for c in range(4):
        b3a[32 * c:32 * (c + 1), 0] = b3[c]
    d["b3a"] = b3a
    b3b = np.zeros((128, 1), np.float32)
    b3b[64:96, 0] = b3[5]
    b3b[96:128, 0] = b3[4]
    d["b3b"] = b3b
    w4f = np.zeros((128, 12), np.float32)
    for c in range(4):
        w4f[32 * c:32 * (c + 1), c] = W4[c, 0, :]
    w4f[64:96, 6 + 5] = W4[5, 0, :]
    w4f[96:128, 6 + 4] = W4[4, 0, :]
    d["w4f"] = w4f.astype(bf16)
    cstv = np.zeros((128, 8), np.float32)
    cstv[:, 0] = HALF_PI
    for c in range(6):
        cstv[:, 1 + c] = 0.5 * b4[c, 0]
    d["cst"] = cstv
    d["_b4"] = np.ascontiguousarray(b4[:, 0])                  # host-only
    return d


def build_program(R=R_CORE, slice_rows=8192):
    """Build + bacc-compile the program (cached)."""
    key = (R, slice_rows)
    if key in _BUILD_CACHE:
        return _BUILD_CACHE[key]
    nc, emit = _build(R, slice_rows)
    emit()
    nc.compile()
    _BUILD_CACHE[key] = nc
    return nc


def kernel(res, W1, b1, W2, b2, W3, b3, W4, b4, coupling):
    """Full-input entry point: shards res over 8 cores, runs the SPMD
    kernel, gathers and returns (act, raw) like the reference."""
    from concourse.bass_utils import run_bass_kernel_spmd

    res = np.ascontiguousarray(np.asarray(res, np.float32))
    W1 = np.asarray(W1, np.float32); b1 = np.asarray(b1, np.float32)
    W2 = np.asarray(W2, np.float32); b2 = np.asarray(b2, np.float32)
    W3 = np.asarray(W3, np.float32); b3 = np.asarray(b3, np.float32)
    W4 = np.asarray(W4, np.float32); b4 = np.asarray(b4, np.float32)

    wd = prep_weights(W1, b1, W2, b2, W3, b3, W4, b4)
    b4vec = wd.pop("_b4")
    nc = build_program(R_CORE, 8192)

    import ml_dtypes
    res_pad = np.zeros((B, 128), ml_dtypes.bfloat16)
    res_pad[:, :RES_DIM] = res.astype(ml_dtypes.bfloat16)
    in_maps = []
    for i in range(N_CORES):
        m = dict(wd)
        m["res_pad"] = np.ascontiguousarray(res_pad[i * R_CORE:(i + 1) * R_CORE])
        in_maps.append(m)
    out = run_bass_kernel_spmd(nc, in_maps, list(range(N_CORES)))
    act = np.concatenate([out.results[i]["act_o"] for i in range(N_CORES)], axis=0)
    raw = np.concatenate([out.results[i]["raw_o"] for i in range(N_CORES)], axis=0)
    raw = raw + b4vec[None, :]
    return act.astype(np.float32), raw.astype(np.float32)


# revision 13
# speedup vs baseline: 1.7156x; 1.0244x over previous
"""Trainium2 Bass kernel for nn_Chambers: 6 per-chamber MLPs over a shared
reservoir input, followed by 5 coupled-chamber fixed-point iterations.

Data-parallel over 8 NeuronCores: each core processes B/8 = 32768 rows.

Per-core pipeline (feature-major MLP, batch-major coupling):
  - res [R,100] loaded in [128,100] row tiles, transposed on the PE
    (is_transpose matmul vs identity) into resT [100, N] in PSUM, copied
    to SBUF by the DVE.
  - L1 (K=100,M=128) per chamber; silu+bias fused into one ACT op
    reading PSUM (bias is a per-partition AP).
  - L2 (K=128,M=64): two chambers run concurrently via column tiling
    (tile_position (0,0)/(0,64)), one [128,N] PSUM tile -> one silu op.
  - L3 (K=64,M=32): six chambers concurrent via row+column tiling.
  - L4: per-chamber dot products are K-stacked into two accumulating
    matmuls (block-column lhsT) producing raw = z4 as a [6,N] PSUM tile.
  - raw is DMA-scattered into batch-major [128, 6*F] tiles; sigmoid is
    computed as 0.5+0.5*tanh(0.5*x+0.5*b4) so every ACT function used
    (Silu/Tanh/Sin) lives in the single `silu_and_others` table set.
  - 5 coupling iterations run on the DVE using sin/cos expansion:
      delta_i = K*( cos a_i * (C sin a)_i - sin a_i * (C cos a)_i )
    with the 6x6 symmetric matvec done as 15 paired
    scalar_tensor_tensor AXPYs ([2,F] strided block-pair APs).
  - Coupling runs per batch slice so it overlaps the next slice's MLP.

Outputs: act (post-coupling) and raw (z4; b4 added on host).
"""

import numpy as np

# ---- problem constants (fixed by the task; kernel.py must be self-contained)
B = 262144
RES_DIM = 100
NCH = 6
CF_ITERS = 5
CF_K = 0.02
DECAY = np.array([0.9, 0.93, 0.85, 0.97, 0.88, 0.94], dtype=np.float32)
COUPLING = np.array([
    [0.0, -0.3, 0.6, 0.4, -0.2, 0.3],
    [-0.3, 0.0, -0.5, -0.7, 0.6, 0.4],
    [0.6, -0.5, 0.0, 0.3, -0.3, 0.2],
    [0.4, -0.7, 0.3, 0.0, -0.4, 0.5],
    [-0.2, 0.6, -0.3, -0.4, 0.0, 0.3],
    [0.3, 0.4, 0.2, 0.5, 0.3, 0.0]], dtype=np.float32)
N_CORES = 8
R_CORE = B // N_CORES          # 32768 rows per core
CHUNK = 2048                   # rows per MLP chunk
HALF_PI = float(np.pi / 2.0)

_BUILD_CACHE = {}


def _build(R, slice_rows):
    """Emit + compile the per-core SPMD program for R rows, coupling in
    slices of slice_rows. Returns the compiled Bacc object."""
    from contextlib import ExitStack
    import concourse.bass as bass
    import concourse.mybir as mybir
    from concourse import bacc, tile, masks

    f32 = mybir.dt.float32
    AF = mybir.ActivationFunctionType
    OP = mybir.AluOpType

    assert R % slice_rows == 0 and slice_rows % CHUNK == 0
    n_slices = R // slice_rows
    chunks_per_slice = slice_rows // CHUNK
    F = slice_rows // 128          # free cols per chamber in coupling tiles
    PPC = CHUNK // F               # partitions covered by one chunk's scatter
    KC = (CF_K * COUPLING).astype(np.float64)
    # (i,j) pairs: 3 "init" pairs covering each block once, 12 accumulating
    init_pairs = [(0, 1), (2, 3), (4, 5)]
    rest_pairs = [(i, j) for i in range(6) for j in range(i + 1, 6)
                  if (i, j) not in init_pairs]

    nc = bacc.Bacc("TRN2", target_bir_lowering=False, debug=False,
                   num_devices=N_CORES)
    bf16 = mybir.dt.bfloat16
    # res zero-padded to 128 cols, bf16, so the 2-byte xbar DMA-transpose
    # can load it DRAM->SBUF already transposed (128x128 tiles)
    res = nc.dram_tensor("res_pad", [R, 128], bf16, kind="ExternalInput").ap()
    w1t = nc.dram_tensor("w1t", [RES_DIM, 6 * 128], bf16, kind="ExternalInput").ap()
    b1t = nc.dram_tensor("b1t", [128, 6], f32, kind="ExternalInput").ap()
    w2t = nc.dram_tensor("w2t", [128, 6 * 64], bf16, kind="ExternalInput").ap()
    b2p = nc.dram_tensor("b2p", [128, 3], f32, kind="ExternalInput").ap()
    # w3t holds W3^T twice (rows 0-63 and 64-127): row-tiled matmuls need
    # the stationary operand at the same base partition as the moving one
    w3t = nc.dram_tensor("w3t", [128, 6 * 32], bf16, kind="ExternalInput").ap()
    b3a = nc.dram_tensor("b3a", [128, 1], f32, kind="ExternalInput").ap()
    b3b = nc.dram_tensor("b3b", [128, 1], f32, kind="ExternalInput").ap()
    w4f = nc.dram_tensor("w4f", [128, 12], bf16, kind="ExternalInput").ap()
    # cst col 0 = pi/2 (cos bias); cols 1..6 = 0.5*b4[c] (tanh biases)
    cst = nc.dram_tensor("cst", [128, 8], f32, kind="ExternalInput").ap()
    act_o = nc.dram_tensor("act_o", [R, 6], f32, kind="ExternalOutput").ap()
    raw_o = nc.dram_tensor("raw_o", [R, 6], f32, kind="ExternalOutput").ap()

    def emit():
        with tile.TileContext(nc) as tc, ExitStack() as ctx:
            wp = ctx.enter_context(tc.tile_pool(name="w", bufs=1))
            t_w1t = wp.tile([RES_DIM, 6 * 128], bf16, tag="w1t")
            nc.gpsimd.dma_start(t_w1t[:], w1t)
            t_b1t = wp.tile([128, 6], f32, tag="b1t")
            nc.gpsimd.dma_start(t_b1t[:], b1t)
            t_w2t = wp.tile([128, 6 * 64], bf16, tag="w2t")
            nc.gpsimd.dma_start(t_w2t[:], w2t)
            t_b2p = wp.tile([128, 3], f32, tag="b2p")
            nc.gpsimd.dma_start(t_b2p[:], b2p)
            t_w3t = wp.tile([128, 6 * 32], bf16, tag="w3t")
            nc.gpsimd.dma_start(t_w3t[:], w3t)
            t_b3a = wp.tile([128, 1], f32, tag="b3a")
            nc.gpsimd.dma_start(t_b3a[:], b3a)
            t_b3b = wp.tile([128, 1], f32, tag="b3b")
            nc.gpsimd.dma_start(t_b3b[:], b3b)
            t_w4f = wp.tile([128, 12], bf16, tag="w4f")
            nc.gpsimd.dma_start(t_w4f[:], w4f)
            t_cst = wp.tile([128, 8], f32, tag="cst")
            nc.gpsimd.dma_start(t_cst[:], cst)
            p_rT = ctx.enter_context(tc.tile_pool(name="rT", bufs=3))
            p_mm = ctx.enter_context(tc.tile_pool(name="pmm", bufs=2, space="PSUM"))
            p_h1 = ctx.enter_context(tc.tile_pool(name="h1", bufs=3))
            p_h2 = ctx.enter_context(tc.tile_pool(name="h2", bufs=4))
            p_h3 = ctx.enter_context(tc.tile_pool(name="h3", bufs=2))
            p_rsb = ctx.enter_context(tc.tile_pool(name="rsb", bufs=2))
            p_bm = ctx.enter_context(tc.tile_pool(name="bm", bufs=2))
            p_cpl = ctx.enter_context(tc.tile_pool(name="cpl", bufs=2))
            p_out = ctx.enter_context(tc.tile_pool(name="out", bufs=2))

            def blk(ap_t, c):
                return ap_t[:, c * F:(c + 1) * F]

            def pair_out(ap_t, i, j):
                x3 = ap_t[:].rearrange("p (c f) -> p c f", f=F)
                return x3[:, i:j + 1:(j - i), :]

            def pair_src(ap_t, i, j):
                # blocks [j, i] (swapped) via negative step
                x3 = ap_t[:].rearrange("p (c f) -> p c f", f=F)
                if i == 0:
                    return x3[:, j::-(j - i), :][:, 0:2, :]
                return x3[:, j:i - 1:-(j - i), :]

            for s in range(n_slices):
                rawbm = p_bm.tile([128, 6 * F], f32, tag="rawbm")
                for k in range(chunks_per_slice):
                    g = s * chunks_per_slice + k       # global chunk id
                    # -- load res transposed via the 2-byte xbar DMA
                    # (rows 100..127 hold transposed zero padding)
                    rTf = p_rT.tile([128, CHUNK], bf16, tag="rT")
                    rT = rTf[0:RES_DIM, :]
                    nc.sync.dma_start(
                        rTf[:],
                        res[g * CHUNK:(g + 1) * CHUNK, :],
                        transpose=True)

                    # -- L1 + L2 (paired)
                    h1s = {}
                    h2s = {}
                    for c in range(6):
                        ps1 = p_mm.tile([128, CHUNK], f32, tag="mm")
                        for h in range(CHUNK // 512):
                            nc.tensor.matmul(
                                ps1[:, h * 512:(h + 1) * 512],
                                t_w1t[:, c * 128:(c + 1) * 128],
                                rT[:, h * 512:(h + 1) * 512])
                        h1 = p_h1.tile([128, CHUNK], bf16, tag="h1")
                        nc.scalar.activation(h1[:], ps1[:], AF.Silu,
                                             bias=t_b1t[:, c:c + 1])
                        h1s[c] = h1
                        if c % 2 == 1:
                            p = c // 2
                            ps2 = p_mm.tile([128, CHUNK], f32, tag="mm")
                            for h in range(CHUNK // 512):
                                nc.tensor.matmul(
                                    ps2[0:64, h * 512:(h + 1) * 512],
                                    t_w2t[:, (2 * p) * 64:(2 * p + 1) * 64],
                                    h1s[2 * p][:, h * 512:(h + 1) * 512],
                                    tile_position=(0, 0))
                                nc.tensor.matmul(
                                    ps2[64:128, h * 512:(h + 1) * 512],
                                    t_w2t[:, (2 * p + 1) * 64:(2 * p + 2) * 64],
                                    h1s[2 * p + 1][:, h * 512:(h + 1) * 512],
                                    tile_position=(0, 64))
                            h2 = p_h2.tile([128, CHUNK], bf16, tag="h2")
                            nc.scalar.activation(h2[:], ps2[:], AF.Silu,
                                                 bias=t_b2p[:, p:p + 1])
                            h2s[p] = h2

                    # -- L3: chambers 0-3 -> ps3a rows 0..127; c5 -> ps3b rows
                    # 64..95, c4 -> ps3b rows 96..127 (row base for K=64 must
                    # be 0/64; all six (row,col) subarray sets stay distinct)
                    ps3a = p_mm.tile([128, CHUNK], f32, tag="mm")
                    ps3b = p_mm.tile([128, CHUNK], f32, tag="mm")
                    # (chamber, h2 pair, rows(0=upper 64,1=lower), out tile, out row base)
                    l3 = [(0, 0, 0, ps3a, 0), (1, 0, 1, ps3a, 32),
                          (2, 1, 0, ps3a, 64), (3, 1, 1, ps3a, 96),
                          (4, 2, 0, ps3b, 96), (5, 2, 1, ps3b, 64)]
                    for (c, p, half, pst, rb) in l3:
                        for h in range(CHUNK // 512):
                            nc.tensor.matmul(
                                pst[rb:rb + 32, h * 512:(h + 1) * 512],
                                t_w3t[64 * half:64 * half + 64, c * 32:(c + 1) * 32],
                                h2s[p][64 * half:64 * half + 64,
                                       h * 512:(h + 1) * 512],
                                tile_position=(64 * half, rb))
                    h3a = p_h3.tile([128, CHUNK], bf16, tag="h3a")
                    nc.scalar.activation(h3a[:], ps3a[:], AF.Silu, bias=t_b3a[:])
                    h3b = p_h3.tile([128, CHUNK], bf16, tag="h3b")
                    nc.scalar.activation(h3b[64:128, :], ps3b[64:128, :], AF.Silu,
                                         bias=t_b3b[64:128, :])

                    # -- L4: raw[0:6] = w4f.T @ h3 (K-stacked accumulation)
                    for h in range(CHUNK // 512):
                        nc.tensor.matmul(
                            ps3b[0:6, h * 512:(h + 1) * 512],
                            t_w4f[:, 0:6],
                            h3a[:, h * 512:(h + 1) * 512],
                            start=True, stop=False, tile_position=(0, 0))
                        nc.tensor.matmul(
                            ps3b[0:6, h * 512:(h + 1) * 512],
                            t_w4f[64:128, 6:12],
                            h3b[64:128, h * 512:(h + 1) * 512],
                            start=False, stop=True, tile_position=(64, 0))
                    rsb = p_rsb.tile([6, CHUNK], f32, tag="rsb")
                    nc.vector.tensor_copy(rsb[:], ps3b[0:6, :])
                    # -- scatter into batch-major rawbm [128, 6F]
                    for c in range(6):
                        nc.gpsimd.dma_start(
                            rawbm[k * PPC:(k + 1) * PPC, c * F:(c + 1) * F],
                            rsb[c:c + 1, :].rearrange("o (a f) -> o a f", f=F))

                # ---- coupling for slice s (batch-major [128, 6F] tiles)
                tt = p_cpl.tile([128, 6 * F], f32, tag="T")
                for c in range(6):
                    nc.scalar.activation(blk(tt, c), blk(rawbm, c), AF.Tanh,
                                         bias=t_cst[:, 1 + c:2 + c], scale=0.5)
                A = p_cpl.tile([128, 6 * F], f32, tag="A")
                nc.vector.tensor_scalar(A[:], tt[:], 0.5, 0.5, OP.mult, OP.add)
                for it in range(CF_ITERS):
                    D = p_cpl.tile([128, 6 * F], f32, tag="D")
                    for c in range(6):
                        nc.vector.tensor_scalar_mul(blk(D, c), blk(A, c),
                                                    float(DECAY[c]))
                    SN = p_cpl.tile([128, 6 * F], f32, tag="SN")
                    nc.scalar.activation(SN[:], D[:], AF.Sin)
                    CS = p_cpl.tile([128, 6 * F], f32, tag="CS")
                    nc.scalar.activation(CS[:], D[:], AF.Sin, bias=t_cst[:, 0:1])
                    P = p_cpl.tile([128, 6 * F], f32, tag="P")
                    Q = p_cpl.tile([128, 6 * F], f32, tag="Q")
                    for (i, j) in init_pairs:
                        nc.vector.tensor_scalar(pair_out(P, i, j),
                                                pair_src(SN, i, j),
                                                float(KC[i][j]), None, OP.mult)
                        nc.vector.tensor_scalar(pair_out(Q, i, j),
                                                pair_src(CS, i, j),
                                                float(KC[i][j]), None, OP.mult)
                    for (i, j) in rest_pairs:
                        nc.vector.scalar_tensor_tensor(
                            pair_out(P, i, j), pair_src(SN, i, j),
                            float(KC[i][j]), pair_out(P, i, j),
                            OP.mult, OP.add)
                        nc.vector.scalar_tensor_tensor(
                            pair_out(Q, i, j), pair_src(CS, i, j),
                            float(KC[i][j]), pair_out(Q, i, j),
                            OP.mult, OP.add)
                    U1 = p_cpl.tile([128, 6 * F], f32, tag="U1")
                    nc.vector.tensor_tensor(U1[:], CS[:], P[:], OP.mult)
                    U2 = p_cpl.tile([128, 6 * F], f32, tag="U2")
                    nc.vector.tensor_tensor(U2[:], SN[:], Q[:], OP.mult)
                    DD = p_cpl.tile([128, 6 * F], f32, tag="DD")
                    nc.vector.tensor_tensor(DD[:], U1[:], U2[:], OP.subtract)
                    V = p_cpl.tile([128, 6 * F], f32, tag="V")
                    nc.vector.tensor_tensor(V[:], D[:], DD[:], OP.add)
                    A = p_cpl.tile([128, 6 * F], f32, tag="A")
                    nc.vector.tensor_scalar(A[:], V[:], 0.0, 1.0, OP.max, OP.min)

                # ---- outputs: interleave [p, c*F+f] -> [p, f*6+c], then DMA
                OA = p_out.tile([128, 6 * F], f32, tag="oa")
                nc.vector.tensor_copy(
                    OA[:].rearrange("p (f c) -> p c f", c=6),
                    A[:].rearrange("p (c f) -> p c f", f=F))
                nc.gpsimd.dma_start(
                    act_o[s * slice_rows:(s + 1) * slice_rows, :]
                    .rearrange("(p x) c -> p (x c)", p=128),
                    OA[:])
                OR = p_out.tile([128, 6 * F], f32, tag="orr")
                nc.vector.tensor_copy(
                    OR[:].rearrange("p (f c) -> p c f", c=6),
                    rawbm[:].rearrange("p (c f) -> p c f", f=F))
                nc.gpsimd.dma_start(
                    raw_o[s * slice_rows:(s + 1) * slice_rows, :]
                    .rearrange("(p x) c -> p (x c)", p=128),
                    OR[:])
    return nc, emit


def prep_weights(W1, b1, W2, b2, W3, b3, W4, b4):
    """Host-side weight layout preparation."""
    import ml_dtypes
    bf16 = ml_dtypes.bfloat16
    d = {}
    d["w1t"] = np.ascontiguousarray(
        W1.transpose(2, 0, 1).reshape(RES_DIM, 6 * 128)).astype(bf16)
    d["b1t"] = np.ascontiguousarray(b1.T)                      # [128, 6]
    d["w2t"] = np.ascontiguousarray(
        W2.transpose(2, 0, 1).reshape(128, 6 * 64)).astype(bf16)
    b2p = np.zeros((128, 3), np.float32)
    for p in range(3):
        b2p[0:64, p] = b2[2 * p]
        b2p[64:128, p] = b2[2 * p + 1]
    d["b2p"] = b2p
    w3t_h = W3.transpose(2, 0, 1).reshape(64, 6 * 32)
    d["w3t"] = np.ascontiguousarray(
        np.concatenate([w3t_h, w3t_h], axis=0)).astype(bf16)
    b3a = np.zeros((128, 1), np.float32)
    for c in range(4):
        b3a[32 * c:32 * (c + 1), 0] = b3[c]
    d["b3a"] = b3a
    b3b = np.zeros((128, 1), np.float32)
    b3b[64:96, 0] = b3[5]
    b3b[96:128, 0] = b3[4]
    d["b3b"] = b3b
    w4f = np.zeros((128, 12), np.float32)
    for c in range(4):
        w4f[32 * c:32 * (c + 1), c] = W4[c, 0, :]
    w4f[64:96, 6 + 5] = W4[5, 0, :]
    w4f[96:128, 6 + 4] = W4[4, 0, :]
    d["w4f"] = w4f.astype(bf16)
    cstv = np.zeros((128, 8), np.float32)
    cstv[:, 0] = HALF_PI
    for c in range(6):
        cstv[:, 1 + c] = 0.5 * b4[c, 0]
    d["cst"] = cstv
    d["_b4"] = np.ascontiguousarray(b4[:, 0])                  # host-only
    return d


def build_program(R=R_CORE, slice_rows=8192):
    """Build + bacc-compile the program (cached)."""
    key = (R, slice_rows)
    if key in _BUILD_CACHE:
        return _BUILD_CACHE[key]
    nc, emit = _build(R, slice_rows)
    emit()
    nc.compile()
    _BUILD_CACHE[key] = nc
    return nc


def kernel(res, W1, b1, W2, b2, W3, b3, W4, b4, coupling):
    """Full-input entry point: shards res over 8 cores, runs the SPMD
    kernel, gathers and returns (act, raw) like the reference."""
    from concourse.bass_utils import run_bass_kernel_spmd

    res = np.ascontiguousarray(np.asarray(res, np.float32))
    W1 = np.asarray(W1, np.float32); b1 = np.asarray(b1, np.float32)
    W2 = np.asarray(W2, np.float32); b2 = np.asarray(b2, np.float32)
    W3 = np.asarray(W3, np.float32); b3 = np.asarray(b3, np.float32)
    W4 = np.asarray(W4, np.float32); b4 = np.asarray(b4, np.float32)

    wd = prep_weights(W1, b1, W2, b2, W3, b3, W4, b4)
    b4vec = wd.pop("_b4")
    nc = build_program(R_CORE, 8192)

    import ml_dtypes
    res_pad = np.zeros((B, 128), ml_dtypes.bfloat16)
    res_pad[:, :RES_DIM] = res.astype(ml_dtypes.bfloat16)
    in_maps = []
    for i in range(N_CORES):
        m = dict(wd)
        m["res_pad"] = np.ascontiguousarray(res_pad[i * R_CORE:(i + 1) * R_CORE])
        in_maps.append(m)
    out = run_bass_kernel_spmd(nc, in_maps, list(range(N_CORES)))
    act = np.concatenate([out.results[i]["act_o"] for i in range(N_CORES)], axis=0)
    raw = np.concatenate([out.results[i]["raw_o"] for i in range(N_CORES)], axis=0)
    raw = raw + b4vec[None, :]
    return act.astype(np.float32), raw.astype(np.float32)


# revision 17
# speedup vs baseline: 1.9045x; 1.1101x over previous
"""Trainium2 Bass kernel for nn_Chambers: 6 per-chamber MLPs over a shared
reservoir input, followed by 5 coupled-chamber fixed-point iterations.

Data-parallel over 8 NeuronCores: each core processes B/8 = 32768 rows.

Per-core pipeline (feature-major MLP, batch-major coupling):
  - res [R,100] loaded in [128,100] row tiles, transposed on the PE
    (is_transpose matmul vs identity) into resT [100, N] in PSUM, copied
    to SBUF by the DVE.
  - L1 (K=100,M=128) per chamber; silu+bias fused into one ACT op
    reading PSUM (bias is a per-partition AP).
  - L2 (K=128,M=64): two chambers run concurrently via column tiling
    (tile_position (0,0)/(0,64)), one [128,N] PSUM tile -> one silu op.
  - L3 (K=64,M=32): six chambers concurrent via row+column tiling.
  - L4: per-chamber dot products are K-stacked into two accumulating
    matmuls (block-column lhsT) producing raw = z4 as a [6,N] PSUM tile.
  - raw is DMA-scattered into batch-major [128, 6*F] tiles; sigmoid is
    computed as 0.5+0.5*tanh(0.5*x+0.5*b4) so every ACT function used
    (Silu/Tanh/Sin) lives in the single `silu_and_others` table set.
  - 5 coupling iterations run on the DVE using sin/cos expansion:
      delta_i = K*( cos a_i * (C sin a)_i - sin a_i * (C cos a)_i )
    with the 6x6 symmetric matvec done as 15 paired
    scalar_tensor_tensor AXPYs ([2,F] strided block-pair APs).
  - Coupling runs per batch slice so it overlaps the next slice's MLP.

Outputs: act (post-coupling) and raw (z4; b4 added on host).
"""

import numpy as np

# ---- problem constants (fixed by the task; kernel.py must be self-contained)
B = 262144
RES_DIM = 100
NCH = 6
CF_ITERS = 5
CF_K = 0.02
DECAY = np.array([0.9, 0.93, 0.85, 0.97, 0.88, 0.94], dtype=np.float32)
COUPLING = np.array([
    [0.0, -0.3, 0.6, 0.4, -0.2, 0.3],
    [-0.3, 0.0, -0.5, -0.7, 0.6, 0.4],
    [0.6, -0.5, 0.0, 0.3, -0.3, 0.2],
    [0.4, -0.7, 0.3, 0.0, -0.4, 0.5],
    [-0.2, 0.6, -0.3, -0.4, 0.0, 0.3],
    [0.3, 0.4, 0.2, 0.5, 0.3, 0.0]], dtype=np.float32)
N_CORES = 8
R_CORE = B // N_CORES          # 32768 rows per core
CHUNK = 2048                   # rows per MLP chunk
HALF_PI = float(np.pi / 2.0)

_BUILD_CACHE = {}


def _build(R, slice_sizes):
    """Emit + compile the per-core SPMD program for R rows, coupling in
    slices of slice_sizes[i] rows (descending sizes shrink the serial
    coupling tail). Returns the compiled Bacc object."""
    from contextlib import ExitStack
    import concourse.bass as bass
    import concourse.mybir as mybir
    from concourse import bacc, tile, masks

    f32 = mybir.dt.float32
    AF = mybir.ActivationFunctionType
    OP = mybir.AluOpType

    assert sum(slice_sizes) == R and all(s % CHUNK == 0 for s in slice_sizes)
    KC = (CF_K * COUPLING).astype(np.float64)
    # (i,j) pairs: 3 "init" pairs covering each block once, 12 accumulating
    init_pairs = [(0, 1), (2, 3), (4, 5)]
    rest_pairs = [(i, j) for i in range(6) for j in range(i + 1, 6)
                  if (i, j) not in init_pairs]

    nc = bacc.Bacc("TRN2", target_bir_lowering=False, debug=False,
                   num_devices=N_CORES)
    bf16 = mybir.dt.bfloat16
    # res zero-padded to 128 cols, bf16, so the 2-byte xbar DMA-transpose
    # can load it DRAM->SBUF already transposed (128x128 tiles)
    res = nc.dram_tensor("res_pad", [R, 128], bf16, kind="ExternalInput").ap()
    w1t = nc.dram_tensor("w1t", [RES_DIM, 6 * 128], bf16, kind="ExternalInput").ap()
    b1t = nc.dram_tensor("b1t", [128, 6], f32, kind="ExternalInput").ap()
    w2t = nc.dram_tensor("w2t", [128, 6 * 64], bf16, kind="ExternalInput").ap()
    b2p = nc.dram_tensor("b2p", [128, 3], f32, kind="ExternalInput").ap()
    # w3t holds W3^T twice (rows 0-63 and 64-127): row-tiled matmuls need
    # the stationary operand at the same base partition as the moving one
    w3t = nc.dram_tensor("w3t", [128, 6 * 32], bf16, kind="ExternalInput").ap()
    b3a = nc.dram_tensor("b3a", [128, 1], f32, kind="ExternalInput").ap()
    b3b = nc.dram_tensor("b3b", [128, 1], f32, kind="ExternalInput").ap()
    w4f = nc.dram_tensor("w4f", [128, 12], bf16, kind="ExternalInput").ap()
    # cst col 0 = pi/2 (cos bias); cols 1..6 = 0.5*b4[c] (tanh biases)
    cst = nc.dram_tensor("cst", [128, 8], f32, kind="ExternalInput").ap()
    act_o = nc.dram_tensor("act_o", [R, 6], f32, kind="ExternalOutput").ap()
    raw_o = nc.dram_tensor("raw_o", [R, 6], f32, kind="ExternalOutput").ap()

    def emit():
        with tile.TileContext(nc) as tc, ExitStack() as ctx:
            wp = ctx.enter_context(tc.tile_pool(name="w", bufs=1))
            t_w1t = wp.tile([RES_DIM, 6 * 128], bf16, tag="w1t")
            nc.gpsimd.dma_start(t_w1t[:], w1t)
            t_b1t = wp.tile([128, 6], f32, tag="b1t")
            nc.gpsimd.dma_start(t_b1t[:], b1t)
            t_w2t = wp.tile([128, 6 * 64], bf16, tag="w2t")
            nc.gpsimd.dma_start(t_w2t[:], w2t)
            t_b2p = wp.tile([128, 3], f32, tag="b2p")
            nc.gpsimd.dma_start(t_b2p[:], b2p)
            t_w3t = wp.tile([128, 6 * 32], bf16, tag="w3t")
            nc.gpsimd.dma_start(t_w3t[:], w3t)
            t_b3a = wp.tile([128, 1], f32, tag="b3a")
            nc.gpsimd.dma_start(t_b3a[:], b3a)
            t_b3b = wp.tile([128, 1], f32, tag="b3b")
            nc.gpsimd.dma_start(t_b3b[:], b3b)
            t_w4f = wp.tile([128, 12], bf16, tag="w4f")
            nc.gpsimd.dma_start(t_w4f[:], w4f)
            t_cst = wp.tile([128, 8], f32, tag="cst")
            nc.gpsimd.dma_start(t_cst[:], cst)
            p_rT = ctx.enter_context(tc.tile_pool(name="rT", bufs=3))
            p_mm = ctx.enter_context(tc.tile_pool(name="pmm", bufs=4, space="PSUM"))
            p_h1 = ctx.enter_context(tc.tile_pool(name="h1", bufs=3))
            p_h2 = ctx.enter_context(tc.tile_pool(name="h2", bufs=4))
            p_h3 = ctx.enter_context(tc.tile_pool(name="h3", bufs=2))
            p_rsb = ctx.enter_context(tc.tile_pool(name="rsb", bufs=2))
            p_bm = ctx.enter_context(tc.tile_pool(name="bm", bufs=2))
            p_cpl = ctx.enter_context(tc.tile_pool(name="cpl", bufs=2))
            p_out = ctx.enter_context(tc.tile_pool(name="out", bufs=2))

            def blk(ap_t, c, F):
                return ap_t[:, c * F:(c + 1) * F]

            def pair_out(ap_t, i, j, F):
                x3 = ap_t[:].rearrange("p (c f) -> p c f", f=F)
                return x3[:, i:j + 1:(j - i), :]

            def pair_src(ap_t, i, j, F):
                # blocks [j, i] (swapped) via negative step
                x3 = ap_t[:].rearrange("p (c f) -> p c f", f=F)
                if i == 0:
                    return x3[:, j::-(j - i), :][:, 0:2, :]
                return x3[:, j:i - 1:-(j - i), :]

            FMAX = max(slice_sizes) // 128
            g0 = 0
            for s, srows in enumerate(slice_sizes):
                F = srows // 128
                PPC = CHUNK // F
                chunks_per_slice = srows // CHUNK
                rawbm = p_bm.tile([128, 6 * FMAX], f32, tag="rawbm")
                for k in range(chunks_per_slice):
                    g = g0 + k                         # global chunk id
                    # -- load res transposed via the 2-byte xbar DMA
                    # (rows 100..127 hold transposed zero padding)
                    rTf = p_rT.tile([128, CHUNK], bf16, tag="rT")
                    rT = rTf[0:RES_DIM, :]
                    nc.sync.dma_start(
                        rTf[:],
                        res[g * CHUNK:(g + 1) * CHUNK, :],
                        transpose=True)

                    # -- L1 + L2 (paired)
                    h1s = {}
                    h2s = {}
                    for c in range(6):
                        h1 = p_h1.tile([128, CHUNK], bf16, tag="h1")
                        for q in range(CHUNK // 1024):
                            ps1 = p_mm.tile([128, 1024], f32, tag="mm")
                            for h in range(2):
                                nc.tensor.matmul(
                                    ps1[:, h * 512:(h + 1) * 512],
                                    t_w1t[:, c * 128:(c + 1) * 128],
                                    rT[:, (2 * q + h) * 512:(2 * q + h + 1) * 512])
                            nc.scalar.activation(
                                h1[:, q * 1024:(q + 1) * 1024], ps1[:],
                                AF.Silu, bias=t_b1t[:, c:c + 1])
                        h1s[c] = h1
                        if c % 2 == 1:
                            p = c // 2
                            h2 = p_h2.tile([128, CHUNK], bf16, tag="h2")
                            for q in range(CHUNK // 1024):
                                ps2 = p_mm.tile([128, 1024], f32, tag="mm")
                                for h in range(2):
                                    o = (2 * q + h) * 512
                                    nc.tensor.matmul(
                                        ps2[0:64, h * 512:(h + 1) * 512],
                                        t_w2t[:, (2 * p) * 64:(2 * p + 1) * 64],
                                        h1s[2 * p][:, o:o + 512],
                                        tile_position=(0, 0))
                                    nc.tensor.matmul(
                                        ps2[64:128, h * 512:(h + 1) * 512],
                                        t_w2t[:, (2 * p + 1) * 64:(2 * p + 2) * 64],
                                        h1s[2 * p + 1][:, o:o + 512],
                                        tile_position=(0, 64))
                                nc.scalar.activation(
                                    h2[:, q * 1024:(q + 1) * 1024], ps2[:],
                                    AF.Silu, bias=t_b2p[:, p:p + 1])
                            h2s[p] = h2

                    # -- L3: chambers 0-3 -> ps3a rows 0..127; c5 -> ps3b rows
                    # 64..95, c4 -> ps3b rows 96..127 (row base for K=64 must
                    # be 0/64; all six (row,col) subarray sets stay distinct)
                    h3a = p_h3.tile([128, CHUNK], bf16, tag="h3a")
                    h3b = p_h3.tile([128, CHUNK], bf16, tag="h3b")
                    rsb = p_rsb.tile([6, CHUNK], f32, tag="rsb")
                    # (chamber, h2 pair, rows(0=upper 64,1=lower), a-tile?, row base)
                    l3 = [(0, 0, 0, True, 0), (1, 0, 1, True, 32),
                          (2, 1, 0, True, 64), (3, 1, 1, True, 96),
                          (4, 2, 0, False, 96), (5, 2, 1, False, 64)]
                    for q in range(CHUNK // 1024):
                        ps3a = p_mm.tile([128, 1024], f32, tag="mm")
                        ps3b = p_mm.tile([128, 1024], f32, tag="mm")
                        qq = slice(q * 1024, (q + 1) * 1024)
                        for (c, p, half, in_a, rb) in l3:
                            pst = ps3a if in_a else ps3b
                            for h in range(2):
                                o = (2 * q + h) * 512
                                nc.tensor.matmul(
                                    pst[rb:rb + 32, h * 512:(h + 1) * 512],
                                    t_w3t[64 * half:64 * half + 64,
                                          c * 32:(c + 1) * 32],
                                    h2s[p][64 * half:64 * half + 64, o:o + 512],
                                    tile_position=(64 * half, rb))
                        nc.scalar.activation(h3a[:, qq], ps3a[:], AF.Silu,
                                             bias=t_b3a[:])
                        nc.scalar.activation(h3b[64:128, qq], ps3b[64:128, :],
                                             AF.Silu, bias=t_b3b[64:128, :])
                        # -- L4: raw[0:6] accumulating K-stacked matmuls
                        for h in range(2):
                            o = (2 * q + h) * 512
                            nc.tensor.matmul(
                                ps3b[0:6, h * 512:(h + 1) * 512],
                                t_w4f[:, 0:6],
                                h3a[:, o:o + 512],
                                start=True, stop=False, tile_position=(0, 0))
                            nc.tensor.matmul(
                                ps3b[0:6, h * 512:(h + 1) * 512],
                                t_w4f[64:128, 6:12],
                                h3b[64:128, o:o + 512],
                                start=False, stop=True, tile_position=(64, 0))
                        nc.vector.tensor_copy(rsb[:, qq], ps3b[0:6, :])
                    # -- scatter into batch-major rawbm [128, 6F]
                    for c in range(6):
                        nc.gpsimd.dma_start(
                            rawbm[k * PPC:(k + 1) * PPC, c * F:(c + 1) * F],
                            rsb[c:c + 1, :].rearrange("o (a f) -> o a f", f=F))

                # ---- coupling for slice s (batch-major [128, 6F] tiles)
                def ctile(tag):
                    t = p_cpl.tile([128, 6 * FMAX], f32, tag=tag)
                    return t[:, 0:6 * F]

                tt = ctile("T")
                for c in range(6):
                    nc.scalar.activation(blk(tt, c, F), blk(rawbm, c, F), AF.Tanh,
                                         bias=t_cst[:, 1 + c:2 + c], scale=0.5)
                A = ctile("A")
                nc.vector.tensor_scalar(A[:], tt[:], 0.5, 0.5, OP.mult, OP.add)
                for it in range(CF_ITERS):
                    D = ctile("D")
                    for c in range(6):
                        nc.vector.tensor_scalar_mul(blk(D, c, F), blk(A, c, F),
                                                    float(DECAY[c]))
                    SN = ctile("SN")
                    nc.scalar.activation(SN[:], D[:], AF.Sin)
                    CS = ctile("CS")
                    nc.scalar.activation(CS[:], D[:], AF.Sin, bias=t_cst[:, 0:1])
                    P = ctile("P")
                    Q = ctile("Q")
                    for (i, j) in init_pairs:
                        nc.vector.tensor_scalar(pair_out(P, i, j, F),
                                                pair_src(SN, i, j, F),
                                                float(KC[i][j]), None, OP.mult)
                        nc.vector.tensor_scalar(pair_out(Q, i, j, F),
                                                pair_src(CS, i, j, F),
                                                float(KC[i][j]), None, OP.mult)
                    for (i, j) in rest_pairs:
                        nc.vector.scalar_tensor_tensor(
                            pair_out(P, i, j, F), pair_src(SN, i, j, F),
                            float(KC[i][j]), pair_out(P, i, j, F),
                            OP.mult, OP.add)
                        nc.vector.scalar_tensor_tensor(
                            pair_out(Q, i, j, F), pair_src(CS, i, j, F),
                            float(KC[i][j]), pair_out(Q, i, j, F),
                            OP.mult, OP.add)
                    U1 = ctile("U1")
                    nc.vector.tensor_tensor(U1[:], CS[:], P[:], OP.mult)
                    U2 = ctile("U2")
                    nc.vector.tensor_tensor(U2[:], SN[:], Q[:], OP.mult)
                    DD = ctile("DD")
                    nc.vector.tensor_tensor(DD[:], U1[:], U2[:], OP.subtract)
                    V = ctile("V")
                    nc.vector.tensor_tensor(V[:], D[:], DD[:], OP.add)
                    A = ctile("A")
                    nc.vector.tensor_scalar(A[:], V[:], 0.0, 1.0, OP.max, OP.min)

                # ---- outputs: interleave [p, c*F+f] -> [p, f*6+c], then DMA
                r0 = g0 * CHUNK
                OA = p_out.tile([128, 6 * FMAX], f32, tag="oa")
                nc.vector.tensor_copy(
                    OA[:, 0:6 * F].rearrange("p (f c) -> p c f", c=6),
                    A[:].rearrange("p (c f) -> p c f", f=F))
                nc.gpsimd.dma_start(
                    act_o[r0:r0 + srows, :]
                    .rearrange("(p x) c -> p (x c)", p=128),
                    OA[:, 0:6 * F])
                OR = p_out.tile([128, 6 * FMAX], f32, tag="orr")
                nc.vector.tensor_copy(
                    OR[:, 0:6 * F].rearrange("p (f c) -> p c f", c=6),
                    rawbm[:, 0:6 * F].rearrange("p (c f) -> p c f", f=F))
                nc.gpsimd.dma_start(
                    raw_o[r0:r0 + srows, :]
                    .rearrange("(p x) c -> p (x c)", p=128),
                    OR[:, 0:6 * F])
                g0 += chunks_per_slice
    return nc, emit


def prep_weights(W1, b1, W2, b2, W3, b3, W4, b4):
    """Host-side weight layout preparation."""
    import ml_dtypes
    bf16 = ml_dtypes.bfloat16
    d = {}
    d["w1t"] = np.ascontiguousarray(
        W1.transpose(2, 0, 1).reshape(RES_DIM, 6 * 128)).astype(bf16)
    d["b1t"] = np.ascontiguousarray(b1.T)                      # [128, 6]
    d["w2t"] = np.ascontiguousarray(
        W2.transpose(2, 0, 1).reshape(128, 6 * 64)).astype(bf16)
    b2p = np.zeros((128, 3), np.float32)
    for p in range(3):
        b2p[0:64, p] = b2[2 * p]
        b2p[64:128, p] = b2[2 * p + 1]
    d["b2p"] = b2p
    w3t_h = W3.transpose(2, 0, 1).reshape(64, 6 * 32)
    d["w3t"] = np.ascontiguousarray(
        np.concatenate([w3t_h, w3t_h], axis=0)).astype(bf16)
    b3a = np.zeros((128, 1), np.float32)
    for c in range(4):
        b3a[32 * c:32 * (c + 1), 0] = b3[c]
    d["b3a"] = b3a
    b3b = np.zeros((128, 1), np.float32)
    b3b[64:96, 0] = b3[5]
    b3b[96:128, 0] = b3[4]
    d["b3b"] = b3b
    w4f = np.zeros((128, 12), np.float32)
    for c in range(4):
        w4f[32 * c:32 * (c + 1), c] = W4[c, 0, :]
    w4f[64:96, 6 + 5] = W4[5, 0, :]
    w4f[96:128, 6 + 4] = W4[4, 0, :]
    d["w4f"] = w4f.astype(bf16)
    cstv = np.zeros((128, 8), np.float32)
    cstv[:, 0] = HALF_PI
    for c in range(6):
        cstv[:, 1 + c] = 0.5 * b4[c, 0]
    d["cst"] = cstv
    d["_b4"] = np.ascontiguousarray(b4[:, 0])                  # host-only
    return d


DEFAULT_SLICES = (8192, 8192, 8192, 4096, 2048, 2048)


def build_program(R=R_CORE, slice_sizes=DEFAULT_SLICES):
    """Build + bacc-compile the program (cached)."""
    key = (R, tuple(slice_sizes))
    if key in _BUILD_CACHE:
        return _BUILD_CACHE[key]
    nc, emit = _build(R, list(slice_sizes))
    emit()
    nc.compile()
    _BUILD_CACHE[key] = nc
    return nc


def kernel(res, W1, b1, W2, b2, W3, b3, W4, b4, coupling):
    """Full-input entry point: shards res over 8 cores, runs the SPMD
    kernel, gathers and returns (act, raw) like the reference."""
    from concourse.bass_utils import run_bass_kernel_spmd

    res = np.ascontiguousarray(np.asarray(res, np.float32))
    W1 = np.asarray(W1, np.float32); b1 = np.asarray(b1, np.float32)
    W2 = np.asarray(W2, np.float32); b2 = np.asarray(b2, np.float32)
    W3 = np.asarray(W3, np.float32); b3 = np.asarray(b3, np.float32)
    W4 = np.asarray(W4, np.float32); b4 = np.asarray(b4, np.float32)

    wd = prep_weights(W1, b1, W2, b2, W3, b3, W4, b4)
    b4vec = wd.pop("_b4")
    nc = build_program(R_CORE)

    import ml_dtypes
    res_pad = np.zeros((B, 128), ml_dtypes.bfloat16)
    res_pad[:, :RES_DIM] = res.astype(ml_dtypes.bfloat16)
    in_maps = []
    for i in range(N_CORES):
        m = dict(wd)
        m["res_pad"] = np.ascontiguousarray(res_pad[i * R_CORE:(i + 1) * R_CORE])
        in_maps.append(m)
    out = run_bass_kernel_spmd(nc, in_maps, list(range(N_CORES)))
    act = np.concatenate([out.results[i]["act_o"] for i in range(N_CORES)], axis=0)
    raw = np.concatenate([out.results[i]["raw_o"] for i in range(N_CORES)], axis=0)
    raw = raw + b4vec[None, :]
    return act.astype(np.float32), raw.astype(np.float32)
